# revision 1
# baseline (speedup 1.0000x reference)
"""GAT influence layer on 8 Trainium2 NeuronCores (Bass/Tile).

Strategy (edge-parallel, row-sharded):
  Pass 1 (device): each core computes its 12.5k-node slice of
      Wh = h @ W, s_src = Wh @ a_src, s_dst = Wh @ a_dst
      via TensorE matmuls against an augmented weight matrix.
  Host: replicates/permutes device-computed Wh into per-core edge-slot
      streams (edges bucketed by destination-node block, 128-edge tiles),
      plus per-slot q_src/q_dst/row_rel arrays.  Data movement only.
  Pass 2 (device): per edge tile: exp(leakyrelu(q_src+q_dst)) on ACT/DVE,
      message scaling on ACT, one-hot selection matrix on DVE, and the
      softmax-weighted segment-sum as PSUM-accumulated TensorE matmuls;
      deferred division by the per-node denominator (the global max-
      subtract of the reference cancels analytically in the softmax).
  Host: concatenates per-core node-partitioned outputs.
"""

import os
import numpy as np

N_NODES = 100000
N_EDGES = 1600000
IN_DIM = 128
OUT_DIM = 64
NEG_SLOPE = 0.2
CORES = 8
NPC = N_NODES // CORES          # nodes per core (12500)
BW = 64                         # nodes per block (matmul window)
BPC = (NPC + BW - 1) // BW      # blocks per core (196)
NPP = BPC * BW                  # padded nodes per core (12544)
RT = NPP // 128                 # pass-1 row tiles per core (98)
SBB = 14                        # blocks per superblock (196/14 = 14 sbs)
PAD_Q = -30000.0                # pad-slot attention logit -> exp == 0

LAST_STATS = {}


def _build_pass1():
    from concourse import bacc, mybir
    import concourse.tile as tile

    f32 = mybir.dt.float32
    nc = bacc.Bacc("TRN2", target_bir_lowering=False, debug=False)
    d_hT = nc.dram_tensor("hT", [128, NPP], f32, kind="ExternalInput")
    d_W = nc.dram_tensor("Wm", [IN_DIM, OUT_DIM], f32, kind="ExternalInput")
    d_WT = nc.dram_tensor("WT", [OUT_DIM, IN_DIM], f32, kind="ExternalInput")
    d_a2 = nc.dram_tensor("a2", [OUT_DIM, 2], f32, kind="ExternalInput")
    d_wh = nc.dram_tensor("wh", [NPP, OUT_DIM], f32, kind="ExternalOutput")
    d_s = nc.dram_tensor("sp", [128, RT * 2], f32, kind="ExternalOutput")

    with tile.TileContext(nc) as tc:
        with tc.tile_pool(name="c1", bufs=1) as cp, \
             tc.tile_pool(name="sp1", bufs=4) as sp, \
             tc.tile_pool(name="wo1", bufs=4) as wo, \
             tc.tile_pool(name="ps1", bufs=4, space="PSUM") as psp:
            w_sb = cp.tile([IN_DIM, OUT_DIM], f32)
            nc.sync.dma_start(out=w_sb[:], in_=d_W[:])
            wt_sb = cp.tile([OUT_DIM, IN_DIM], f32)
            nc.sync.dma_start(out=wt_sb[:], in_=d_WT[:])
            a_sb = cp.tile([OUT_DIM, 2], f32)
            nc.sync.dma_start(out=a_sb[:], in_=d_a2[:])

            waug = cp.tile([IN_DIM, OUT_DIM + 2], f32)
            nc.vector.tensor_copy(out=waug[:, 0:OUT_DIM], in_=w_sb[:])
            ws_ps = psp.tile([IN_DIM, 2], f32, space="PSUM")
            nc.tensor.matmul(out=ws_ps[:], lhsT=wt_sb[:], rhs=a_sb[:],
                             start=True, stop=True)
            nc.vector.tensor_copy(out=waug[:, OUT_DIM:OUT_DIM + 2], in_=ws_ps[:])

            s_stage = cp.tile([128, RT * 2], f32)
            for r in range(RT):
                ht = sp.tile([128, 128], f32)
                nc.sync.dma_start(out=ht[:], in_=d_hT[:, r * 128:(r + 1) * 128])
                wh_ps = psp.tile([128, OUT_DIM + 2], f32, space="PSUM")
                nc.tensor.matmul(out=wh_ps[:], lhsT=ht[:], rhs=waug[:],
                                 start=True, stop=True)
                wh_sb = wo.tile([128, OUT_DIM], f32)
                nc.vector.tensor_copy(out=wh_sb[:], in_=wh_ps[:, 0:OUT_DIM])
                nc.scalar.copy(out=s_stage[:, 2 * r:2 * r + 2],
                               in_=wh_ps[:, OUT_DIM:OUT_DIM + 2])
                nc.sync.dma_start(out=d_wh[r * 128:(r + 1) * 128, :], in_=wh_sb[:])
            nc.sync.dma_start(out=d_s[:], in_=s_stage[:])
    nc.compile()
    return nc


def _build_pass2(Tj, Ttot):
    from concourse import bacc, mybir
    import concourse.tile as tile

    f32 = mybir.dt.float32
    i32 = mybir.dt.int32
    alu = mybir.AluOpType
    act = mybir.ActivationFunctionType
    W65 = OUT_DIM + 1

    base = np.zeros(BPC + 1, np.int64)
    base[1:] = np.cumsum(Tj)
    assert base[-1] == Ttot

    nc = bacc.Bacc("TRN2", target_bir_lowering=False, debug=False)
    d_msg = nc.dram_tensor("msg", [128, Ttot * W65], f32, kind="ExternalInput")
    d_qs = nc.dram_tensor("qs", [128, Ttot], f32, kind="ExternalInput")
    d_qd = nc.dram_tensor("qd", [128, Ttot], f32, kind="ExternalInput")
    d_rr = nc.dram_tensor("rr", [128, Ttot], f32, kind="ExternalInput")
    d_out = nc.dram_tensor("out", [NPP, OUT_DIM], f32, kind="ExternalOutput")

    n_sb = BPC // SBB
    with tile.TileContext(nc) as tc:
        with tc.tile_pool(name="c2", bufs=1) as cp, \
             tc.tile_pool(name="gp", bufs=2) as gp, \
             tc.tile_pool(name="xp", bufs=6) as xp, \
             tc.tile_pool(name="mp", bufs=6) as mp, \
             tc.tile_pool(name="fp", bufs=6) as fp, \
             tc.tile_pool(name="op", bufs=2) as op, \
             tc.tile_pool(name="pp", bufs=6, space="PSUM") as pp:

            iota_i = cp.tile([128, BW], i32)
            nc.gpsimd.iota(iota_i[:], pattern=[[1, BW]], base=0, channel_multiplier=0)
            iota_f = cp.tile([128, BW], f32)
            nc.vector.tensor_copy(out=iota_f[:], in_=iota_i[:])

            qs_sb = cp.tile([128, Ttot], f32)
            nc.sync.dma_start(out=qs_sb[:], in_=d_qs[:])
            qd_sb = cp.tile([128, Ttot], f32)
            nc.sync.dma_start(out=qd_sb[:], in_=d_qd[:])
            rr_sb = cp.tile([128, Ttot], f32)
            nc.sync.dma_start(out=rr_sb[:], in_=d_rr[:])

            # exp(leakyrelu(qs + qd)) for every slot, one shot
            ex_sb = cp.tile([128, Ttot], f32)
            nc.vector.tensor_tensor(out=ex_sb[:], in0=qs_sb[:], in1=qd_sb[:], op=alu.add)
            sc_sb = cp.tile([128, Ttot], f32)
            nc.vector.tensor_scalar(out=sc_sb[:], in0=ex_sb[:], scalar1=NEG_SLOPE,
                                    scalar2=None, op0=alu.mult)
            nc.vector.tensor_tensor(out=ex_sb[:], in0=ex_sb[:], in1=sc_sb[:], op=alu.max)
            nc.scalar.activation(out=ex_sb[:], in_=ex_sb[:], func=act.Exp)

            for s in range(n_sb):
                j0, j1 = s * SBB, (s + 1) * SBB
                t0, t1 = int(base[j0]), int(base[j1])
                T_s = t1 - t0
                G = gp.tile([128, T_s * W65], f32, tag="G")
                nc.sync.dma_start(out=G[:], in_=d_msg[:, t0 * W65:t1 * W65])
                out_stage = op.tile([BW, SBB * BW], f32, tag="ost")
                for j in range(j0, j1):
                    tj = int(Tj[j])
                    ps = pp.tile([BW, W65], f32, space="PSUM", tag="ps")
                    for t in range(tj):
                        tc_g = int(base[j]) + t
                        rel = tc_g - t0
                        X = xp.tile([128, W65], f32, tag="X")
                        nc.scalar.activation(
                            out=X[:], in_=G[:, rel * W65:(rel + 1) * W65],
                            func=act.Copy, scale=ex_sb[:, tc_g:tc_g + 1])
                        M = mp.tile([128, BW], f32, tag="M")
                        nc.vector.tensor_scalar(
                            out=M[:], in0=iota_f[:], scalar1=rr_sb[:, tc_g:tc_g + 1],
                            scalar2=None, op0=alu.is_equal)
                        nc.tensor.matmul(out=ps[:], lhsT=M[:], rhs=X[:],
                                         start=(t == 0), stop=(t == tj - 1))
                    dtmp = fp.tile([BW, 1], f32, tag="dt")
                    nc.vector.tensor_scalar(out=dtmp[:], in0=ps[:, OUT_DIM:W65],
                                            scalar1=1e-10, scalar2=None, op0=alu.add)
                    dinv = fp.tile([BW, 1], f32, tag="di")
                    nc.vector.reciprocal(out=dinv[:], in_=dtmp[:])
                    jr = j - j0
                    nc.vector.tensor_scalar(
                        out=out_stage[:, jr * BW:(jr + 1) * BW],
                        in0=ps[:, 0:OUT_DIM], scalar1=dinv[:],
                        scalar2=None, op0=alu.mult)
                out_ap = d_out[j0 * BW:j1 * BW, :].rearrange("(b p) f -> p b f", p=BW)
                in_ap = out_stage[:].rearrange("p (b f) -> p b f", b=SBB)
                nc.sync.dma_start(out=out_ap, in_=in_ap)
    nc.compile()
    return nc


def _prep_structure(row, col):
    """Bucket edges by (core, dest-node block); assign each edge a slot
    (partition p, tile column t) in its block's 128-edge tiles."""
    core = row // NPC
    nloc = row - core * NPC
    blk = nloc // BW
    gblk = core * BPC + blk
    cnt = np.bincount(gblk, minlength=CORES * BPC)
    Tj = np.maximum(1, (cnt.reshape(CORES, BPC).max(axis=0) + 127) // 128)
    base = np.zeros(BPC + 1, np.int64)
    base[1:] = np.cumsum(Tj)
    Ttot = int(base[-1])

    order = np.argsort(gblk, kind="stable")
    starts = np.zeros(CORES * BPC, np.int64)
    starts[1:] = np.cumsum(cnt)[:-1]
    rank = np.arange(N_EDGES, dtype=np.int64) - np.repeat(starts, cnt)
    gblk_s = gblk[order]
    core_s = gblk_s // BPC
    blk_s = gblk_s - core_s * BPC
    t_loc = rank >> 7
    p_s = rank & 127
    tglob = base[blk_s] + t_loc
    return dict(order=order, core_s=core_s, blk_s=blk_s, p_s=p_s, tglob=tglob,
                Tj=Tj, base=base, Ttot=Ttot)


def _run_spmd(nc, in_maps, trace=False):
    from concourse import bass_utils
    res = bass_utils.run_bass_kernel_spmd(
        nc, in_maps, core_ids=list(range(CORES)), trace=trace)
    return res


def kernel(h, row, col, W, a):
    trace = bool(os.environ.get("GAT_TRACE"))
    if trace:
        try:
            import ntff_shim
            ntff_shim.install()
        except Exception:
            trace = False

    h = np.ascontiguousarray(np.asarray(h, dtype=np.float32))
    W = np.ascontiguousarray(np.asarray(W, dtype=np.float32))
    a = np.ascontiguousarray(np.asarray(a, dtype=np.float32)).reshape(2 * OUT_DIM)
    row = np.asarray(row).astype(np.int64)
    col = np.asarray(col).astype(np.int64)

    # ---- pass 1: Wh / s_src / s_dst, node-sharded ----
    nc1 = _build_pass1()
    WT = np.ascontiguousarray(W.T)
    a2 = np.ascontiguousarray(np.stack([a[:OUT_DIM], a[OUT_DIM:]], axis=1))
    in_maps1 = []
    for c in range(CORES):
        hpad = np.zeros((NPP, IN_DIM), np.float32)
        hpad[:NPC] = h[c * NPC:(c + 1) * NPC]
        in_maps1.append({"hT": np.ascontiguousarray(hpad.T), "Wm": W,
                         "WT": WT, "a2": a2})
    res1 = _run_spmd(nc1, in_maps1, trace=trace)
    if trace:
        LAST_STATS["pass1_ns"] = res1.exec_time_ns

    WhA = np.ones((N_NODES, OUT_DIM + 1), np.float32)
    s_src = np.empty(N_NODES, np.float32)
    s_dst = np.empty(N_NODES, np.float32)
    for c in range(CORES):
        WhA[c * NPC:(c + 1) * NPC, :OUT_DIM] = res1.results[c]["wh"][:NPC]
        s3 = res1.results[c]["sp"].reshape(128, RT, 2).transpose(1, 0, 2).reshape(NPP, 2)
        s_src[c * NPC:(c + 1) * NPC] = s3[:NPC, 0]
        s_dst[c * NPC:(c + 1) * NPC] = s3[:NPC, 1]

    # ---- host: edge-slot structure + replicated-Wh message streams ----
    st = _prep_structure(row, col)
    Tj, Ttot = st["Tj"], st["Ttot"]
    W65 = OUT_DIM + 1
    cs, ps, tg = st["core_s"], st["p_s"], st["tglob"]
    row_s = row[st["order"]]
    col_s = col[st["order"]]

    msg = np.zeros((CORES, 128, Ttot, W65), np.float32)
    msg[cs, ps, tg] = WhA[col_s]
    qs = np.full((CORES, 128, Ttot), PAD_Q, np.float32)
    qs[cs, ps, tg] = s_src[row_s]
    qd = np.full((CORES, 128, Ttot), PAD_Q, np.float32)
    qd[cs, ps, tg] = s_dst[col_s]
    rr = np.zeros((CORES, 128, Ttot), np.float32)
    rr[cs, ps, tg] = (row_s - cs * NPC - st["blk_s"] * BW).astype(np.float32)

    # ---- pass 2: attention + segment sum ----
    nc2 = _build_pass2(Tj, Ttot)
    in_maps2 = [{"msg": msg[c].reshape(128, Ttot * W65), "qs": qs[c],
                 "qd": qd[c], "rr": rr[c]} for c in range(CORES)]
    res2 = _run_spmd(nc2, in_maps2, trace=trace)
    if trace:
        LAST_STATS["pass2_ns"] = res2.exec_time_ns
        LAST_STATS["total_ns"] = (res1.exec_time_ns or 0) + (res2.exec_time_ns or 0)

    out = np.empty((N_NODES, OUT_DIM), np.float32)
    for c in range(CORES):
        out[c * NPC:(c + 1) * NPC] = res2.results[c]["out"][:NPC]
    return out


# revision 6
# speedup vs baseline: 1.9751x; 1.9751x over previous
"""GAT influence layer on 8 Trainium2 NeuronCores (Bass/Tile).

Strategy (edge-parallel, row-sharded):
  Pass 1 (device): each core computes its 12.5k-node slice of
      Wh = h @ W, s_src = Wh @ a_src, s_dst = Wh @ a_dst
      via TensorE matmuls against an augmented weight matrix.
  Host: replicates/permutes device-computed Wh into per-core edge-slot
      streams (edges bucketed by destination-node block, 128-edge tiles),
      plus per-slot q_src/q_dst/row_rel arrays.  Data movement only.
  Pass 2 (device): one-shot exp(leakyrelu(q_src+q_dst)); per superblock a
      batched exp-weighted one-hot selection matrix (two tensor_tensor ops
      with broadcast APs, alternating DVE/GpSimd); the softmax-weighted
      segment-sum as PSUM-accumulated TensorE matmuls over the message
      stream; deferred division by the per-node denominator (the global
      max-subtract of the reference cancels analytically in the softmax).
  Host: concatenates per-core node-partitioned outputs.
"""

import os
import numpy as np

N_NODES = 100000
N_EDGES = 1600000
IN_DIM = 128
OUT_DIM = 64
NEG_SLOPE = 0.2
CORES = 8
NPC = N_NODES // CORES          # nodes per core (12500)
BW = 64                         # nodes per block (matmul window)
BPC = (NPC + BW - 1) // BW      # blocks per core (196)
NPP = BPC * BW                  # padded nodes per core (12544)
RT = NPP // 128                 # pass-1 row tiles per core (98)
SBB = 7                         # blocks per superblock (196/7 = 28 sbs)
PAD_Q = -30000.0                # pad-slot attention logit -> exp == 0

LAST_STATS = {}


def _build_pass1():
    from concourse import bacc, mybir
    import concourse.tile as tile

    f32 = mybir.dt.float32
    nc = bacc.Bacc("TRN2", target_bir_lowering=False, debug=False)
    d_hT = nc.dram_tensor("hT", [128, NPP], f32, kind="ExternalInput")
    d_W = nc.dram_tensor("Wm", [IN_DIM, OUT_DIM], f32, kind="ExternalInput")
    d_WT = nc.dram_tensor("WT", [OUT_DIM, IN_DIM], f32, kind="ExternalInput")
    d_a2 = nc.dram_tensor("a2", [OUT_DIM, 2], f32, kind="ExternalInput")
    d_wh = nc.dram_tensor("wh", [NPP, OUT_DIM], f32, kind="ExternalOutput")
    d_s = nc.dram_tensor("sp", [128, RT * 2], f32, kind="ExternalOutput")

    with tile.TileContext(nc) as tc:
        with tc.tile_pool(name="c1", bufs=1) as cp, \
             tc.tile_pool(name="psw", bufs=1, space="PSUM") as psw, \
             tc.tile_pool(name="ps1", bufs=6, space="PSUM") as psp:
            ht_sb = cp.tile([128, NPP], f32)
            nc.sync.dma_start(out=ht_sb[:], in_=d_hT[:])
            w_sb = cp.tile([IN_DIM, OUT_DIM], f32)
            nc.sync.dma_start(out=w_sb[:], in_=d_W[:])
            wt_sb = cp.tile([OUT_DIM, IN_DIM], f32)
            nc.sync.dma_start(out=wt_sb[:], in_=d_WT[:])
            a_sb = cp.tile([OUT_DIM, 2], f32)
            nc.sync.dma_start(out=a_sb[:], in_=d_a2[:])

            waug = cp.tile([IN_DIM, OUT_DIM + 2], f32)
            nc.vector.tensor_copy(out=waug[:, 0:OUT_DIM], in_=w_sb[:])
            ws_ps = psw.tile([IN_DIM, 2], f32, space="PSUM")
            nc.tensor.matmul(out=ws_ps[:], lhsT=wt_sb[:], rhs=a_sb[:],
                             start=True, stop=True)
            nc.vector.tensor_copy(out=waug[:, OUT_DIM:OUT_DIM + 2], in_=ws_ps[:])

            s_stage = cp.tile([128, RT * 2], f32)
            wh_stage = cp.tile([128, RT * OUT_DIM], f32)
            for r in range(RT):
                wh_ps = psp.tile([128, OUT_DIM + 2], f32, space="PSUM")
                nc.tensor.matmul(out=wh_ps[:], lhsT=ht_sb[:, r * 128:(r + 1) * 128],
                                 rhs=waug[:], start=True, stop=True)
                nc.vector.tensor_copy(out=wh_stage[:, r * OUT_DIM:(r + 1) * OUT_DIM],
                                      in_=wh_ps[:, 0:OUT_DIM])
                nc.scalar.copy(out=s_stage[:, 2 * r:2 * r + 2],
                               in_=wh_ps[:, OUT_DIM:OUT_DIM + 2])
            wh_ap = d_wh[:].rearrange("(r p) f -> p r f", p=128)
            nc.sync.dma_start(out=wh_ap,
                              in_=wh_stage[:].rearrange("p (r f) -> p r f", r=RT))
            nc.sync.dma_start(out=d_s[:], in_=s_stage[:])
    nc.compile()
    return nc


def _build_pass2(Tj, Ttot):
    from concourse import bacc, mybir
    import concourse.tile as tile

    f32 = mybir.dt.float32
    i32 = mybir.dt.int32
    alu = mybir.AluOpType
    act = mybir.ActivationFunctionType
    W65 = OUT_DIM + 1

    base = np.zeros(BPC + 1, np.int64)
    base[1:] = np.cumsum(Tj)
    assert base[-1] == Ttot

    nc = bacc.Bacc("TRN2", target_bir_lowering=False, debug=False)
    d_msg = nc.dram_tensor("msg", [128, Ttot * W65], f32, kind="ExternalInput")
    d_qs = nc.dram_tensor("qs", [128, Ttot], f32, kind="ExternalInput")
    d_qd = nc.dram_tensor("qd", [128, Ttot], f32, kind="ExternalInput")
    d_rr = nc.dram_tensor("rr", [128, Ttot], f32, kind="ExternalInput")
    d_out = nc.dram_tensor("out", [NPP, OUT_DIM], f32, kind="ExternalOutput")

    n_sb = BPC // SBB
    with tile.TileContext(nc) as tc:
        with tc.tile_pool(name="c2", bufs=1) as cp, \
             tc.tile_pool(name="gp", bufs=3) as gp, \
             tc.tile_pool(name="mp", bufs=3) as mp, \
             tc.tile_pool(name="fp", bufs=8) as fp, \
             tc.tile_pool(name="op", bufs=3) as op, \
             tc.tile_pool(name="pp", bufs=7, space="PSUM") as pp:

            iota_i = cp.tile([128, BW], i32)
            nc.gpsimd.iota(iota_i[:], pattern=[[1, BW]], base=0, channel_multiplier=0)
            iota_f = cp.tile([128, BW], f32)
            nc.vector.tensor_copy(out=iota_f[:], in_=iota_i[:])

            qs_sb = cp.tile([128, Ttot], f32)
            nc.sync.dma_start(out=qs_sb[:], in_=d_qs[:])
            qd_sb = cp.tile([128, Ttot], f32)
            nc.sync.dma_start(out=qd_sb[:], in_=d_qd[:])
            rr_sb = cp.tile([128, Ttot], f32)
            nc.sync.dma_start(out=rr_sb[:], in_=d_rr[:])

            # exp(leakyrelu(qs + qd)) for every slot, one shot
            ex_sb = cp.tile([128, Ttot], f32)
            nc.vector.tensor_tensor(out=ex_sb[:], in0=qs_sb[:], in1=qd_sb[:], op=alu.add)
            sc_sb = cp.tile([128, Ttot], f32)
            nc.vector.tensor_scalar(out=sc_sb[:], in0=ex_sb[:], scalar1=NEG_SLOPE,
                                    scalar2=None, op0=alu.mult)
            nc.vector.tensor_tensor(out=ex_sb[:], in0=ex_sb[:], in1=sc_sb[:], op=alu.max)
            nc.scalar.activation(out=ex_sb[:], in_=ex_sb[:], func=act.Exp)

            for s in range(n_sb):
                j0, j1 = s * SBB, (s + 1) * SBB
                t0, t1 = int(base[j0]), int(base[j1])
                T_s = t1 - t0
                G = gp.tile([128, T_s * W65], f32, tag="G")
                nc.sync.dma_start(out=G[:], in_=d_msg[:, t0 * W65:t1 * W65])

                # M[p, (t,j)] = exp[p,t] * (iota_j == rrel[p,t]) — batched
                M = mp.tile([128, T_s * BW], f32, tag="M")
                eng = nc.vector
                eng.tensor_tensor(
                    out=M[:],
                    in0=iota_f[:].rearrange("p (o f) -> p o f", o=1).to_broadcast([128, T_s, BW]),
                    in1=rr_sb[:, t0:t1].rearrange("p (t o) -> p t o", o=1).to_broadcast([128, T_s, BW]),
                    op=alu.is_equal)
                eng.tensor_tensor(
                    out=M[:], in0=M[:],
                    in1=ex_sb[:, t0:t1].rearrange("p (t o) -> p t o", o=1).to_broadcast([128, T_s, BW]),
                    op=alu.mult)

                out_stage = op.tile([BW, SBB * BW], f32, tag="ost")
                for j in range(j0, j1):
                    tj = int(Tj[j])
                    ps = pp.tile([BW, W65], f32, space="PSUM", tag="ps")
                    for t in range(tj):
                        rel = int(base[j]) + t - t0
                        nc.tensor.matmul(out=ps[:],
                                         lhsT=M[:, rel * BW:(rel + 1) * BW],
                                         rhs=G[:, rel * W65:(rel + 1) * W65],
                                         start=(t == 0), stop=(t == tj - 1))
                    dtmp = fp.tile([BW, 1], f32, tag="dt")
                    nc.vector.tensor_scalar(out=dtmp[:], in0=ps[:, OUT_DIM:W65],
                                            scalar1=1e-10, scalar2=None, op0=alu.add)
                    dinv = fp.tile([BW, 1], f32, tag="di")
                    nc.vector.reciprocal(out=dinv[:], in_=dtmp[:])
                    jr = j - j0
                    nc.scalar.activation(out=out_stage[:, jr * BW:(jr + 1) * BW],
                                         in_=ps[:, 0:OUT_DIM], func=act.Copy,
                                         scale=dinv[:])
                out_ap = d_out[j0 * BW:j1 * BW, :].rearrange("(b p) f -> p b f", p=BW)
                in_ap = out_stage[:].rearrange("p (b f) -> p b f", b=SBB)
                nc.sync.dma_start(out=out_ap, in_=in_ap)
    nc.compile()
    return nc


def _prep_structure(row, col):
    """Bucket edges by (core, dest-node block); assign each edge a slot
    (partition p, tile column t) in its block's 128-edge tiles."""
    core = row // NPC
    nloc = row - core * NPC
    blk = nloc // BW
    gblk = core * BPC + blk
    cnt = np.bincount(gblk, minlength=CORES * BPC)
    Tj = np.maximum(1, (cnt.reshape(CORES, BPC).max(axis=0) + 127) // 128)
    base = np.zeros(BPC + 1, np.int64)
    base[1:] = np.cumsum(Tj)
    Ttot = int(base[-1])

    order = np.argsort(gblk, kind="stable")
    starts = np.zeros(CORES * BPC, np.int64)
    starts[1:] = np.cumsum(cnt)[:-1]
    rank = np.arange(N_EDGES, dtype=np.int64) - np.repeat(starts, cnt)
    gblk_s = gblk[order]
    core_s = gblk_s // BPC
    blk_s = gblk_s - core_s * BPC
    t_loc = rank >> 7
    p_s = rank & 127
    tglob = base[blk_s] + t_loc
    return dict(order=order, core_s=core_s, blk_s=blk_s, p_s=p_s, tglob=tglob,
                Tj=Tj, base=base, Ttot=Ttot)


def _run_spmd(nc, in_maps, trace=False):
    from concourse import bass_utils
    res = bass_utils.run_bass_kernel_spmd(
        nc, in_maps, core_ids=list(range(CORES)), trace=trace)
    return res


def kernel(h, row, col, W, a):
    trace = bool(os.environ.get("GAT_TRACE"))
    if trace:
        try:
            import ntff_shim
            ntff_shim.install()
        except Exception:
            trace = False

    h = np.ascontiguousarray(np.asarray(h, dtype=np.float32))
    W = np.ascontiguousarray(np.asarray(W, dtype=np.float32))
    a = np.ascontiguousarray(np.asarray(a, dtype=np.float32)).reshape(2 * OUT_DIM)
    row = np.asarray(row).astype(np.int64)
    col = np.asarray(col).astype(np.int64)

    # ---- pass 1: Wh / s_src / s_dst, node-sharded ----
    nc1 = _build_pass1()
    WT = np.ascontiguousarray(W.T)
    a2 = np.ascontiguousarray(np.stack([a[:OUT_DIM], a[OUT_DIM:]], axis=1))
    in_maps1 = []
    for c in range(CORES):
        hpad = np.zeros((NPP, IN_DIM), np.float32)
        hpad[:NPC] = h[c * NPC:(c + 1) * NPC]
        in_maps1.append({"hT": np.ascontiguousarray(hpad.T), "Wm": W,
                         "WT": WT, "a2": a2})
    res1 = _run_spmd(nc1, in_maps1, trace=trace)
    if trace:
        LAST_STATS["pass1_ns"] = res1.exec_time_ns

    WhA = np.ones((N_NODES, OUT_DIM + 1), np.float32)
    s_src = np.empty(N_NODES, np.float32)
    s_dst = np.empty(N_NODES, np.float32)
    for c in range(CORES):
        WhA[c * NPC:(c + 1) * NPC, :OUT_DIM] = res1.results[c]["wh"][:NPC]
        s3 = res1.results[c]["sp"].reshape(128, RT, 2).transpose(1, 0, 2).reshape(NPP, 2)
        s_src[c * NPC:(c + 1) * NPC] = s3[:NPC, 0]
        s_dst[c * NPC:(c + 1) * NPC] = s3[:NPC, 1]

    # ---- host: edge-slot structure + replicated-Wh message streams ----
    st = _prep_structure(row, col)
    Tj, Ttot = st["Tj"], st["Ttot"]
    W65 = OUT_DIM + 1
    cs, ps, tg = st["core_s"], st["p_s"], st["tglob"]
    row_s = row[st["order"]]
    col_s = col[st["order"]]

    msg = np.zeros((CORES, 128, Ttot, W65), np.float32)
    msg[cs, ps, tg] = WhA[col_s]
    qs = np.full((CORES, 128, Ttot), PAD_Q, np.float32)
    qs[cs, ps, tg] = s_src[row_s]
    qd = np.full((CORES, 128, Ttot), PAD_Q, np.float32)
    qd[cs, ps, tg] = s_dst[col_s]
    rr = np.zeros((CORES, 128, Ttot), np.float32)
    rr[cs, ps, tg] = (row_s - cs * NPC - st["blk_s"] * BW).astype(np.float32)

    # ---- pass 2: attention + segment sum ----
    nc2 = _build_pass2(Tj, Ttot)
    in_maps2 = [{"msg": msg[c].reshape(128, Ttot * W65), "qs": qs[c],
                 "qd": qd[c], "rr": rr[c]} for c in range(CORES)]
    res2 = _run_spmd(nc2, in_maps2, trace=trace)
    if trace:
        LAST_STATS["pass2_ns"] = res2.exec_time_ns
        LAST_STATS["total_ns"] = (res1.exec_time_ns or 0) + (res2.exec_time_ns or 0)

    out = np.empty((N_NODES, OUT_DIM), np.float32)
    for c in range(CORES):
        out[c * NPC:(c + 1) * NPC] = res2.results[c]["out"][:NPC]
    return out


# revision 9
# speedup vs baseline: 2.0396x; 1.0327x over previous
"""GAT influence layer on 8 Trainium2 NeuronCores (Bass/Tile).

Strategy (edge-parallel, row-sharded):
  Pass 1 (device): each core computes its 12.5k-node slice of
      Wh = h @ W, s_src = Wh @ a_src, s_dst = Wh @ a_dst
      via TensorE matmuls against an augmented weight matrix.
  Host: replicates/permutes device-computed Wh into per-core edge-slot
      streams (edges bucketed by destination-node block, 128-edge tiles),
      plus per-slot q_src/q_dst/row_rel arrays.  Data movement only.
  Pass 2 (device): one-shot exp(leakyrelu(q_src+q_dst)); per superblock a
      batched exp-weighted one-hot selection matrix (two tensor_tensor ops
      with broadcast APs, alternating DVE/GpSimd); the softmax-weighted
      segment-sum as PSUM-accumulated TensorE matmuls over the message
      stream; deferred division by the per-node denominator (the global
      max-subtract of the reference cancels analytically in the softmax).
  Host: concatenates per-core node-partitioned outputs.
"""

import os
import numpy as np

N_NODES = 100000
N_EDGES = 1600000
IN_DIM = 128
OUT_DIM = 64
NEG_SLOPE = 0.2
CORES = 8
NPC = N_NODES // CORES          # nodes per core (12500)
BW = 64                         # nodes per block (matmul window)
BPC = (NPC + BW - 1) // BW      # blocks per core (196)
NPP = BPC * BW                  # padded nodes per core (12544)
RT = NPP // 128                 # pass-1 row tiles per core (98)
SBB = 7                         # blocks per superblock (196/7 = 28 sbs)
PAD_Q = -30000.0                # pad-slot attention logit -> exp == 0

LAST_STATS = {}


def _build_pass1():
    from concourse import bacc, mybir
    import concourse.tile as tile

    f32 = mybir.dt.float32
    nc = bacc.Bacc("TRN2", target_bir_lowering=False, debug=False)
    d_hT = nc.dram_tensor("hT", [128, NPP], f32, kind="ExternalInput")
    d_W = nc.dram_tensor("Wm", [IN_DIM, OUT_DIM], f32, kind="ExternalInput")
    d_WT = nc.dram_tensor("WT", [OUT_DIM, IN_DIM], f32, kind="ExternalInput")
    d_a2 = nc.dram_tensor("a2", [OUT_DIM, 2], f32, kind="ExternalInput")
    d_wh = nc.dram_tensor("wh", [NPP, OUT_DIM], f32, kind="ExternalOutput")
    d_s = nc.dram_tensor("sp", [128, RT * 2], f32, kind="ExternalOutput")

    with tile.TileContext(nc) as tc:
        with tc.tile_pool(name="c1", bufs=1) as cp, \
             tc.tile_pool(name="psw", bufs=1, space="PSUM") as psw, \
             tc.tile_pool(name="ps1", bufs=6, space="PSUM") as psp:
            ht_sb = cp.tile([128, NPP], f32)
            nc.sync.dma_start(out=ht_sb[:], in_=d_hT[:])
            w_sb = cp.tile([IN_DIM, OUT_DIM], f32)
            nc.sync.dma_start(out=w_sb[:], in_=d_W[:])
            wt_sb = cp.tile([OUT_DIM, IN_DIM], f32)
            nc.sync.dma_start(out=wt_sb[:], in_=d_WT[:])
            a_sb = cp.tile([OUT_DIM, 2], f32)
            nc.sync.dma_start(out=a_sb[:], in_=d_a2[:])

            waug = cp.tile([IN_DIM, OUT_DIM + 2], f32)
            nc.vector.tensor_copy(out=waug[:, 0:OUT_DIM], in_=w_sb[:])
            ws_ps = psw.tile([IN_DIM, 2], f32, space="PSUM")
            nc.tensor.matmul(out=ws_ps[:], lhsT=wt_sb[:], rhs=a_sb[:],
                             start=True, stop=True)
            nc.vector.tensor_copy(out=waug[:, OUT_DIM:OUT_DIM + 2], in_=ws_ps[:])

            s_stage = cp.tile([128, RT * 2], f32)
            wh_stage = cp.tile([128, RT * OUT_DIM], f32)
            for r in range(RT):
                wh_ps = psp.tile([128, OUT_DIM + 2], f32, space="PSUM")
                nc.tensor.matmul(out=wh_ps[:], lhsT=ht_sb[:, r * 128:(r + 1) * 128],
                                 rhs=waug[:], start=True, stop=True)
                nc.vector.tensor_copy(out=wh_stage[:, r * OUT_DIM:(r + 1) * OUT_DIM],
                                      in_=wh_ps[:, 0:OUT_DIM])
                nc.scalar.copy(out=s_stage[:, 2 * r:2 * r + 2],
                               in_=wh_ps[:, OUT_DIM:OUT_DIM + 2])
            wh_ap = d_wh[:].rearrange("(r p) f -> p r f", p=128)
            nc.sync.dma_start(out=wh_ap,
                              in_=wh_stage[:].rearrange("p (r f) -> p r f", r=RT))
            nc.sync.dma_start(out=d_s[:], in_=s_stage[:])
    nc.compile()
    return nc


def _build_pass2(Tj, Ttot):
    from concourse import bacc, mybir
    import concourse.tile as tile

    f32 = mybir.dt.float32
    i32 = mybir.dt.int32
    alu = mybir.AluOpType
    act = mybir.ActivationFunctionType
    W65 = OUT_DIM + 1

    base = np.zeros(BPC + 1, np.int64)
    base[1:] = np.cumsum(Tj)
    assert base[-1] == Ttot

    nc = bacc.Bacc("TRN2", target_bir_lowering=False, debug=False)
    d_msg = nc.dram_tensor("msg", [128, Ttot * W65], f32, kind="ExternalInput")
    d_qs = nc.dram_tensor("qs", [128, Ttot], f32, kind="ExternalInput")
    d_qd = nc.dram_tensor("qd", [128, Ttot], f32, kind="ExternalInput")
    d_rr = nc.dram_tensor("rr", [128, Ttot], f32, kind="ExternalInput")
    d_out = nc.dram_tensor("out", [NPP, OUT_DIM], f32, kind="ExternalOutput")

    n_sb = BPC // SBB
    with tile.TileContext(nc) as tc:
        with tc.tile_pool(name="c2", bufs=1) as cp, \
             tc.tile_pool(name="gp", bufs=3) as gp, \
             tc.tile_pool(name="mp", bufs=3) as mp, \
             tc.tile_pool(name="fp", bufs=8) as fp, \
             tc.tile_pool(name="op", bufs=3) as op, \
             tc.tile_pool(name="pp", bufs=7, space="PSUM") as pp:

            iota_i = cp.tile([128, BW], i32)
            nc.gpsimd.iota(iota_i[:], pattern=[[1, BW]], base=0, channel_multiplier=0)
            iota_f = cp.tile([128, BW], f32)
            nc.vector.tensor_copy(out=iota_f[:], in_=iota_i[:])

            qs_sb = cp.tile([128, Ttot], f32)
            nc.sync.dma_start(out=qs_sb[:], in_=d_qs[:])
            qd_sb = cp.tile([128, Ttot], f32)
            nc.sync.dma_start(out=qd_sb[:], in_=d_qd[:])
            rr_sb = cp.tile([128, Ttot], f32)
            nc.sync.dma_start(out=rr_sb[:], in_=d_rr[:])

            # exp(leakyrelu(qs + qd)) for every slot, one shot
            ex_sb = cp.tile([128, Ttot], f32)
            nc.vector.tensor_tensor(out=ex_sb[:], in0=qs_sb[:], in1=qd_sb[:], op=alu.add)
            sc_sb = cp.tile([128, Ttot], f32)
            nc.vector.tensor_scalar(out=sc_sb[:], in0=ex_sb[:], scalar1=NEG_SLOPE,
                                    scalar2=None, op0=alu.mult)
            nc.vector.tensor_tensor(out=ex_sb[:], in0=ex_sb[:], in1=sc_sb[:], op=alu.max)
            nc.scalar.activation(out=ex_sb[:], in_=ex_sb[:], func=act.Exp)

            for s in range(n_sb):
                j0, j1 = s * SBB, (s + 1) * SBB
                t0, t1 = int(base[j0]), int(base[j1])
                T_s = t1 - t0
                G = gp.tile([128, T_s * W65], f32, tag="G")
                nc.sync.dma_start(out=G[:], in_=d_msg[:, t0 * W65:t1 * W65])

                # M[p, (t,j)] = exp[p,t] * (iota_j == rrel[p,t]) — batched
                M = mp.tile([128, T_s * BW], f32, tag="M")
                eng = nc.vector
                eng.tensor_tensor(
                    out=M[:],
                    in0=iota_f[:].rearrange("p (o f) -> p o f", o=1).to_broadcast([128, T_s, BW]),
                    in1=rr_sb[:, t0:t1].rearrange("p (t o) -> p t o", o=1).to_broadcast([128, T_s, BW]),
                    op=alu.is_equal)
                eng.tensor_tensor(
                    out=M[:], in0=M[:],
                    in1=ex_sb[:, t0:t1].rearrange("p (t o) -> p t o", o=1).to_broadcast([128, T_s, BW]),
                    op=alu.mult)

                out_stage = op.tile([BW, SBB * BW], f32, tag="ost")
                for j in range(j0, j1):
                    tj = int(Tj[j])
                    ps = pp.tile([BW, W65], f32, space="PSUM", tag="ps")
                    for t in range(tj):
                        rel = int(base[j]) + t - t0
                        nc.tensor.matmul(out=ps[:],
                                         lhsT=M[:, rel * BW:(rel + 1) * BW],
                                         rhs=G[:, rel * W65:(rel + 1) * W65],
                                         start=(t == 0), stop=(t == tj - 1))
                    dtmp = fp.tile([BW, 1], f32, tag="dt")
                    nc.vector.tensor_scalar(out=dtmp[:], in0=ps[:, OUT_DIM:W65],
                                            scalar1=1e-10, scalar2=None, op0=alu.add)
                    dinv = fp.tile([BW, 1], f32, tag="di")
                    nc.vector.reciprocal(out=dinv[:], in_=dtmp[:])
                    jr = j - j0
                    nc.scalar.activation(out=out_stage[:, jr * BW:(jr + 1) * BW],
                                         in_=ps[:, 0:OUT_DIM], func=act.Copy,
                                         scale=dinv[:])
                out_ap = d_out[j0 * BW:j1 * BW, :].rearrange("(b p) f -> p b f", p=BW)
                in_ap = out_stage[:].rearrange("p (b f) -> p b f", b=SBB)
                nc.sync.dma_start(out=out_ap, in_=in_ap)
    nc.compile()
    return nc


def _prep_structure(row, col):
    """Bucket edges by dest-node 64-block; permute blocks onto (core, slot)
    pairs so that blocks sharing a slot index have similar edge counts
    (shrinks the shared per-slot tile count); assign each edge a slot
    (partition p, tile column t) in its block's 128-edge tiles."""
    NGB = CORES * BPC                       # 1568 block slots (1563 real)
    gb = row // BW                          # global 64-node block per edge
    cnt = np.bincount(gb, minlength=NGB)
    sorted_ids = np.argsort(-cnt, kind="stable")
    blk_core = np.empty(NGB, np.int64)
    blk_slot = np.empty(NGB, np.int64)
    k = np.arange(NGB)
    blk_core[sorted_ids] = k % CORES
    blk_slot[sorted_ids] = k // CORES
    # per slot j: max count over its 8 assigned blocks (sorted -> first of 8)
    Tj = np.maximum(1, (cnt[sorted_ids[::CORES]] + 127) // 128)
    base = np.zeros(BPC + 1, np.int64)
    base[1:] = np.cumsum(Tj)
    Ttot = int(base[-1])

    key = blk_core[gb] * BPC + blk_slot[gb]
    kcnt = np.bincount(key, minlength=NGB)
    order = np.argsort(key, kind="stable")
    starts = np.zeros(NGB, np.int64)
    starts[1:] = np.cumsum(kcnt)[:-1]
    rank = np.arange(N_EDGES, dtype=np.int64) - np.repeat(starts, kcnt)
    key_s = key[order]
    core_s = key_s // BPC
    slot_s = key_s - core_s * BPC
    t_loc = rank >> 7
    p_s = rank & 127
    tglob = base[slot_s] + t_loc
    return dict(order=order, core_s=core_s, p_s=p_s, tglob=tglob,
                gb_s=gb[order], Tj=Tj, base=base, Ttot=Ttot,
                sorted_ids=sorted_ids)


def _run_spmd(nc, in_maps, trace=False):
    from concourse import bass_utils
    res = bass_utils.run_bass_kernel_spmd(
        nc, in_maps, core_ids=list(range(CORES)), trace=trace)
    return res


def kernel(h, row, col, W, a):
    trace = bool(os.environ.get("GAT_TRACE"))
    if trace:
        try:
            import ntff_shim
            ntff_shim.install()
        except Exception:
            trace = False

    h = np.ascontiguousarray(np.asarray(h, dtype=np.float32))
    W = np.ascontiguousarray(np.asarray(W, dtype=np.float32))
    a = np.ascontiguousarray(np.asarray(a, dtype=np.float32)).reshape(2 * OUT_DIM)
    row = np.asarray(row).astype(np.int64)
    col = np.asarray(col).astype(np.int64)

    # ---- pass 1: Wh / s_src / s_dst, node-sharded ----
    nc1 = _build_pass1()
    WT = np.ascontiguousarray(W.T)
    a2 = np.ascontiguousarray(np.stack([a[:OUT_DIM], a[OUT_DIM:]], axis=1))
    in_maps1 = []
    for c in range(CORES):
        hpad = np.zeros((NPP, IN_DIM), np.float32)
        hpad[:NPC] = h[c * NPC:(c + 1) * NPC]
        in_maps1.append({"hT": np.ascontiguousarray(hpad.T), "Wm": W,
                         "WT": WT, "a2": a2})
    res1 = _run_spmd(nc1, in_maps1, trace=trace)
    if trace:
        LAST_STATS["pass1_ns"] = res1.exec_time_ns

    WhA = np.ones((N_NODES, OUT_DIM + 1), np.float32)
    s_src = np.empty(N_NODES, np.float32)
    s_dst = np.empty(N_NODES, np.float32)
    for c in range(CORES):
        WhA[c * NPC:(c + 1) * NPC, :OUT_DIM] = res1.results[c]["wh"][:NPC]
        s3 = res1.results[c]["sp"].reshape(128, RT, 2).transpose(1, 0, 2).reshape(NPP, 2)
        s_src[c * NPC:(c + 1) * NPC] = s3[:NPC, 0]
        s_dst[c * NPC:(c + 1) * NPC] = s3[:NPC, 1]

    # ---- host: edge-slot structure + replicated-Wh message streams ----
    st = _prep_structure(row, col)
    Tj, Ttot = st["Tj"], st["Ttot"]
    W65 = OUT_DIM + 1
    cs, ps, tg = st["core_s"], st["p_s"], st["tglob"]
    row_s = row[st["order"]]
    col_s = col[st["order"]]

    msg = np.zeros((CORES, 128, Ttot, W65), np.float32)
    msg[cs, ps, tg] = WhA[col_s]
    qs = np.full((CORES, 128, Ttot), PAD_Q, np.float32)
    qs[cs, ps, tg] = s_src[row_s]
    qd = np.full((CORES, 128, Ttot), PAD_Q, np.float32)
    qd[cs, ps, tg] = s_dst[col_s]
    rr = np.zeros((CORES, 128, Ttot), np.float32)
    rr[cs, ps, tg] = (row_s - st["gb_s"] * BW).astype(np.float32)

    # ---- pass 2: attention + segment sum ----
    nc2 = _build_pass2(Tj, Ttot)
    in_maps2 = [{"msg": msg[c].reshape(128, Ttot * W65), "qs": qs[c],
                 "qd": qd[c], "rr": rr[c]} for c in range(CORES)]
    res2 = _run_spmd(nc2, in_maps2, trace=trace)
    if trace:
        LAST_STATS["pass2_ns"] = res2.exec_time_ns
        LAST_STATS["total_ns"] = (res1.exec_time_ns or 0) + (res2.exec_time_ns or 0)

    out = np.empty((N_NODES, OUT_DIM), np.float32)
    sorted_ids = st["sorted_ids"]
    NGB_REAL = (N_NODES + BW - 1) // BW
    for c in range(CORES):
        dev = res2.results[c]["out"]
        for j in range(BPC):
            g = int(sorted_ids[j * CORES + c])
            if g >= NGB_REAL:
                continue
            sz = min(BW, N_NODES - g * BW)
            out[g * BW:g * BW + sz] = dev[j * BW:j * BW + sz]
    return out


# revision 12
# speedup vs baseline: 2.1014x; 1.0303x over previous
"""GAT influence layer on 8 Trainium2 NeuronCores (Bass/Tile).

Strategy (edge-parallel, row-sharded):
  Pass 1 (device): each core computes its 12.5k-node slice of
      Wh = h @ W, s_src = Wh @ a_src, s_dst = Wh @ a_dst
      via TensorE matmuls against an augmented weight matrix.
  Host: replicates/permutes device-computed Wh into per-core edge-slot
      streams (edges bucketed by destination-node block, 128-edge tiles),
      plus per-slot q_src/q_dst/row_rel arrays.  Data movement only.
  Pass 2 (device): one-shot exp(leakyrelu(q_src+q_dst)); per superblock a
      batched exp-weighted one-hot selection matrix (two tensor_tensor ops
      with broadcast APs, alternating DVE/GpSimd); the softmax-weighted
      segment-sum as PSUM-accumulated TensorE matmuls over the message
      stream; deferred division by the per-node denominator (the global
      max-subtract of the reference cancels analytically in the softmax).
  Host: concatenates per-core node-partitioned outputs.
"""

import os
import numpy as np

N_NODES = 100000
N_EDGES = 1600000
IN_DIM = 128
OUT_DIM = 64
NEG_SLOPE = 0.2
CORES = 8
NPC = N_NODES // CORES          # nodes per core (12500)
BW = 64                         # nodes per block (matmul window)
BPC = (NPC + BW - 1) // BW      # blocks per core (196)
NPP = BPC * BW                  # padded nodes per core (12544)
RT = NPP // 128                 # pass-1 row tiles per core (98)
SBB = 7                         # blocks per superblock (196/7 = 28 sbs)
PAD_Q = -30000.0                # pad-slot attention logit -> exp == 0

LAST_STATS = {}


def _build_pass1():
    from concourse import bacc, mybir
    import concourse.tile as tile

    f32 = mybir.dt.float32
    nc = bacc.Bacc("TRN2", target_bir_lowering=False, debug=False)
    d_hT = nc.dram_tensor("hT", [128, NPP], f32, kind="ExternalInput")
    d_W = nc.dram_tensor("Wm", [IN_DIM, OUT_DIM], f32, kind="ExternalInput")
    d_WT = nc.dram_tensor("WT", [OUT_DIM, IN_DIM], f32, kind="ExternalInput")
    d_a2 = nc.dram_tensor("a2", [OUT_DIM, 2], f32, kind="ExternalInput")
    d_whT = nc.dram_tensor("whT", [OUT_DIM + 2, NPP], f32, kind="ExternalOutput")

    with tile.TileContext(nc) as tc:
        with tc.tile_pool(name="c1", bufs=1) as cp, \
             tc.tile_pool(name="psw", bufs=1, space="PSUM") as psw, \
             tc.tile_pool(name="ps1", bufs=6, space="PSUM") as psp:
            ht_sb = cp.tile([128, NPP], f32)
            nc.sync.dma_start(out=ht_sb[:], in_=d_hT[:])
            w_sb = cp.tile([IN_DIM, OUT_DIM], f32)
            nc.sync.dma_start(out=w_sb[:], in_=d_W[:])
            wt_sb = cp.tile([OUT_DIM, IN_DIM], f32)
            nc.sync.dma_start(out=wt_sb[:], in_=d_WT[:])
            a_sb = cp.tile([OUT_DIM, 2], f32)
            nc.sync.dma_start(out=a_sb[:], in_=d_a2[:])

            waug = cp.tile([IN_DIM, OUT_DIM + 2], f32)
            nc.vector.tensor_copy(out=waug[:, 0:OUT_DIM], in_=w_sb[:])
            ws_ps = psw.tile([IN_DIM, 2], f32, space="PSUM")
            nc.tensor.matmul(out=ws_ps[:], lhsT=wt_sb[:], rhs=a_sb[:],
                             start=True, stop=True)
            nc.vector.tensor_copy(out=waug[:, OUT_DIM:OUT_DIM + 2], in_=ws_ps[:])

            wh_stage = cp.tile([OUT_DIM + 2, NPP], f32)
            for r in range(RT):
                wh_ps = psp.tile([OUT_DIM + 2, 128], f32, space="PSUM")
                nc.tensor.matmul(out=wh_ps[:], lhsT=waug[:],
                                 rhs=ht_sb[:, r * 128:(r + 1) * 128],
                                 start=True, stop=True)
                nc.vector.tensor_copy(out=wh_stage[:, r * 128:(r + 1) * 128],
                                      in_=wh_ps[:])
            nc.sync.dma_start(out=d_whT[:], in_=wh_stage[:])
    nc.compile()
    return nc


def _build_pass2(Tj, Ttot):
    from concourse import bacc, mybir
    import concourse.tile as tile

    f32 = mybir.dt.float32
    i32 = mybir.dt.int32
    alu = mybir.AluOpType
    act = mybir.ActivationFunctionType
    W65 = OUT_DIM + 1

    base = np.zeros(BPC + 1, np.int64)
    base[1:] = np.cumsum(Tj)
    assert base[-1] == Ttot

    nc = bacc.Bacc("TRN2", target_bir_lowering=False, debug=False)
    d_msg = nc.dram_tensor("msg", [128, Ttot * W65], f32, kind="ExternalInput")
    d_qs = nc.dram_tensor("qs", [128, Ttot], f32, kind="ExternalInput")
    d_qd = nc.dram_tensor("qd", [128, Ttot], f32, kind="ExternalInput")
    d_rr = nc.dram_tensor("rr", [128, Ttot], f32, kind="ExternalInput")
    d_out = nc.dram_tensor("out", [NPP, OUT_DIM], f32, kind="ExternalOutput")

    n_sb = BPC // SBB
    with tile.TileContext(nc) as tc:
        with tc.tile_pool(name="c2", bufs=1) as cp, \
             tc.tile_pool(name="gp", bufs=3) as gp, \
             tc.tile_pool(name="mp", bufs=3) as mp, \
             tc.tile_pool(name="fp", bufs=8) as fp, \
             tc.tile_pool(name="op", bufs=3) as op, \
             tc.tile_pool(name="pp", bufs=7, space="PSUM") as pp:

            iota_i = cp.tile([128, BW], i32)
            nc.gpsimd.iota(iota_i[:], pattern=[[1, BW]], base=0, channel_multiplier=0)
            iota_f = cp.tile([128, BW], f32)
            nc.vector.tensor_copy(out=iota_f[:], in_=iota_i[:])

            qs_sb = cp.tile([128, Ttot], f32)
            nc.sync.dma_start(out=qs_sb[:], in_=d_qs[:])
            qd_sb = cp.tile([128, Ttot], f32)
            nc.sync.dma_start(out=qd_sb[:], in_=d_qd[:])
            rr_sb = cp.tile([128, Ttot], f32)
            nc.sync.dma_start(out=rr_sb[:], in_=d_rr[:])

            # exp(leakyrelu(qs + qd)) for every slot, one shot
            ex_sb = cp.tile([128, Ttot], f32)
            nc.vector.tensor_tensor(out=ex_sb[:], in0=qs_sb[:], in1=qd_sb[:], op=alu.add)
            sc_sb = cp.tile([128, Ttot], f32)
            nc.vector.tensor_scalar(out=sc_sb[:], in0=ex_sb[:], scalar1=NEG_SLOPE,
                                    scalar2=None, op0=alu.mult)
            nc.vector.tensor_tensor(out=ex_sb[:], in0=ex_sb[:], in1=sc_sb[:], op=alu.max)
            nc.scalar.activation(out=ex_sb[:], in_=ex_sb[:], func=act.Exp)

            for s in range(n_sb):
                j0, j1 = s * SBB, (s + 1) * SBB
                t0, t1 = int(base[j0]), int(base[j1])
                T_s = t1 - t0
                G = gp.tile([128, T_s * W65], f32, tag="G")
                nc.sync.dma_start(out=G[:], in_=d_msg[:, t0 * W65:t1 * W65])

                # M[p, (t,j)] = exp[p,t] * (iota_j == rrel[p,t]) — batched
                M = mp.tile([128, T_s * BW], f32, tag="M")
                eng = nc.vector
                eng.tensor_tensor(
                    out=M[:],
                    in0=iota_f[:].rearrange("p (o f) -> p o f", o=1).to_broadcast([128, T_s, BW]),
                    in1=rr_sb[:, t0:t1].rearrange("p (t o) -> p t o", o=1).to_broadcast([128, T_s, BW]),
                    op=alu.is_equal)
                eng.tensor_tensor(
                    out=M[:], in0=M[:],
                    in1=ex_sb[:, t0:t1].rearrange("p (t o) -> p t o", o=1).to_broadcast([128, T_s, BW]),
                    op=alu.mult)

                out_stage = op.tile([BW, SBB * BW], f32, tag="ost")
                for j in range(j0, j1):
                    tj = int(Tj[j])
                    ps = pp.tile([BW, W65], f32, space="PSUM", tag="ps")
                    for t in range(tj):
                        rel = int(base[j]) + t - t0
                        nc.tensor.matmul(out=ps[:],
                                         lhsT=M[:, rel * BW:(rel + 1) * BW],
                                         rhs=G[:, rel * W65:(rel + 1) * W65],
                                         start=(t == 0), stop=(t == tj - 1))
                    dtmp = fp.tile([BW, 1], f32, tag="dt")
                    nc.vector.tensor_scalar(out=dtmp[:], in0=ps[:, OUT_DIM:W65],
                                            scalar1=1e-10, scalar2=None, op0=alu.add)
                    dinv = fp.tile([BW, 1], f32, tag="di")
                    nc.vector.reciprocal(out=dinv[:], in_=dtmp[:])
                    jr = j - j0
                    nc.scalar.activation(out=out_stage[:, jr * BW:(jr + 1) * BW],
                                         in_=ps[:, 0:OUT_DIM], func=act.Copy,
                                         scale=dinv[:])
                out_ap = d_out[j0 * BW:j1 * BW, :].rearrange("(b p) f -> p b f", p=BW)
                in_ap = out_stage[:].rearrange("p (b f) -> p b f", b=SBB)
                nc.sync.dma_start(out=out_ap, in_=in_ap)
    nc.compile()
    return nc


def _prep_structure(row, col):
    """Bucket edges by dest-node 64-block; permute blocks onto (core, slot)
    pairs so that blocks sharing a slot index have similar edge counts
    (shrinks the shared per-slot tile count); assign each edge a slot
    (partition p, tile column t) in its block's 128-edge tiles."""
    NGB = CORES * BPC                       # 1568 block slots (1563 real)
    gb = row // BW                          # global 64-node block per edge
    cnt = np.bincount(gb, minlength=NGB)
    sorted_ids = np.argsort(-cnt, kind="stable")
    blk_core = np.empty(NGB, np.int64)
    blk_slot = np.empty(NGB, np.int64)
    k = np.arange(NGB)
    blk_core[sorted_ids] = k % CORES
    blk_slot[sorted_ids] = k // CORES
    # per slot j: max count over its 8 assigned blocks (sorted -> first of 8)
    Tj = np.maximum(1, (cnt[sorted_ids[::CORES]] + 127) // 128)
    base = np.zeros(BPC + 1, np.int64)
    base[1:] = np.cumsum(Tj)
    Ttot = int(base[-1])

    key = blk_core[gb] * BPC + blk_slot[gb]
    kcnt = np.bincount(key, minlength=NGB)
    order = np.argsort(key, kind="stable")
    starts = np.zeros(NGB, np.int64)
    starts[1:] = np.cumsum(kcnt)[:-1]
    rank = np.arange(N_EDGES, dtype=np.int64) - np.repeat(starts, kcnt)
    key_s = key[order]
    core_s = key_s // BPC
    slot_s = key_s - core_s * BPC
    t_loc = rank >> 7
    p_s = rank & 127
    tglob = base[slot_s] + t_loc
    return dict(order=order, core_s=core_s, p_s=p_s, tglob=tglob,
                gb_s=gb[order], Tj=Tj, base=base, Ttot=Ttot,
                sorted_ids=sorted_ids)


def _run_spmd(nc, in_maps, trace=False):
    from concourse import bass_utils
    res = bass_utils.run_bass_kernel_spmd(
        nc, in_maps, core_ids=list(range(CORES)), trace=trace)
    return res


def kernel(h, row, col, W, a):
    trace = bool(os.environ.get("GAT_TRACE"))
    if trace:
        try:
            import ntff_shim
            ntff_shim.install()
        except Exception:
            trace = False

    h = np.ascontiguousarray(np.asarray(h, dtype=np.float32))
    W = np.ascontiguousarray(np.asarray(W, dtype=np.float32))
    a = np.ascontiguousarray(np.asarray(a, dtype=np.float32)).reshape(2 * OUT_DIM)
    row = np.asarray(row).astype(np.int64)
    col = np.asarray(col).astype(np.int64)

    # ---- pass 1: Wh / s_src / s_dst, node-sharded ----
    nc1 = _build_pass1()
    WT = np.ascontiguousarray(W.T)
    a2 = np.ascontiguousarray(np.stack([a[:OUT_DIM], a[OUT_DIM:]], axis=1))
    in_maps1 = []
    for c in range(CORES):
        hpad = np.zeros((NPP, IN_DIM), np.float32)
        hpad[:NPC] = h[c * NPC:(c + 1) * NPC]
        in_maps1.append({"hT": np.ascontiguousarray(hpad.T), "Wm": W,
                         "WT": WT, "a2": a2})
    res1 = _run_spmd(nc1, in_maps1, trace=trace)
    if trace:
        LAST_STATS["pass1_ns"] = res1.exec_time_ns

    WhA = np.ones((N_NODES, OUT_DIM + 1), np.float32)
    s_src = np.empty(N_NODES, np.float32)
    s_dst = np.empty(N_NODES, np.float32)
    for c in range(CORES):
        whT = res1.results[c]["whT"]
        WhA[c * NPC:(c + 1) * NPC, :OUT_DIM] = whT[:OUT_DIM, :NPC].T
        s_src[c * NPC:(c + 1) * NPC] = whT[OUT_DIM, :NPC]
        s_dst[c * NPC:(c + 1) * NPC] = whT[OUT_DIM + 1, :NPC]

    # ---- host: edge-slot structure + replicated-Wh message streams ----
    st = _prep_structure(row, col)
    Tj, Ttot = st["Tj"], st["Ttot"]
    W65 = OUT_DIM + 1
    cs, ps, tg = st["core_s"], st["p_s"], st["tglob"]
    row_s = row[st["order"]]
    col_s = col[st["order"]]

    msg = np.zeros((CORES, 128, Ttot, W65), np.float32)
    msg[cs, ps, tg] = WhA[col_s]
    qs = np.full((CORES, 128, Ttot), PAD_Q, np.float32)
    qs[cs, ps, tg] = s_src[row_s]
    qd = np.full((CORES, 128, Ttot), PAD_Q, np.float32)
    qd[cs, ps, tg] = s_dst[col_s]
    rr = np.zeros((CORES, 128, Ttot), np.float32)
    rr[cs, ps, tg] = (row_s - st["gb_s"] * BW).astype(np.float32)

    # ---- pass 2: attention + segment sum ----
    nc2 = _build_pass2(Tj, Ttot)
    in_maps2 = [{"msg": msg[c].reshape(128, Ttot * W65), "qs": qs[c],
                 "qd": qd[c], "rr": rr[c]} for c in range(CORES)]
    res2 = _run_spmd(nc2, in_maps2, trace=trace)
    if trace:
        LAST_STATS["pass2_ns"] = res2.exec_time_ns
        LAST_STATS["total_ns"] = (res1.exec_time_ns or 0) + (res2.exec_time_ns or 0)

    out = np.empty((N_NODES, OUT_DIM), np.float32)
    sorted_ids = st["sorted_ids"]
    NGB_REAL = (N_NODES + BW - 1) // BW
    for c in range(CORES):
        dev = res2.results[c]["out"]
        for j in range(BPC):
            g = int(sorted_ids[j * CORES + c])
            if g >= NGB_REAL:
                continue
            sz = min(BW, N_NODES - g * BW)
            out[g * BW:g * BW + sz] = dev[j * BW:j * BW + sz]
    return out


# revision 13
# speedup vs baseline: 2.1540x; 1.0250x over previous
"""GAT influence layer on 8 Trainium2 NeuronCores (Bass/Tile).

Strategy (edge-parallel, row-sharded):
  Pass 1 (device): each core computes its 12.5k-node slice of
      Wh = h @ W, s_src = Wh @ a_src, s_dst = Wh @ a_dst
      via TensorE matmuls against an augmented weight matrix.
  Host: replicates/permutes device-computed Wh into per-core edge-slot
      streams (edges bucketed by destination-node block, 128-edge tiles),
      plus per-slot q_src/q_dst/row_rel arrays.  Data movement only.
  Pass 2 (device): one-shot exp(leakyrelu(q_src+q_dst)); per superblock a
      batched exp-weighted one-hot selection matrix (two tensor_tensor ops
      with broadcast APs, alternating DVE/GpSimd); the softmax-weighted
      segment-sum as PSUM-accumulated TensorE matmuls over the message
      stream; deferred division by the per-node denominator (the global
      max-subtract of the reference cancels analytically in the softmax).
  Host: concatenates per-core node-partitioned outputs.
"""

import os
import numpy as np

N_NODES = 100000
N_EDGES = 1600000
IN_DIM = 128
OUT_DIM = 64
NEG_SLOPE = 0.2
CORES = 8
NPC = N_NODES // CORES          # nodes per core (12500)
BW = 64                         # nodes per block (matmul window)
BPC = (NPC + BW - 1) // BW      # blocks per core (196)
NPP = BPC * BW                  # padded nodes per core (12544)
RT = NPP // 128                 # pass-1 row tiles per core (98)
SBB = 7                         # blocks per superblock (196/7 = 28 sbs)
PAD_Q = -30000.0                # pad-slot attention logit -> exp == 0

LAST_STATS = {}


def _build_pass1():
    from concourse import bacc, mybir
    import concourse.tile as tile

    f32 = mybir.dt.float32
    nc = bacc.Bacc("TRN2", target_bir_lowering=False, debug=False)
    d_hT = nc.dram_tensor("hT", [128, NPP], f32, kind="ExternalInput")
    d_W = nc.dram_tensor("Wm", [IN_DIM, OUT_DIM], f32, kind="ExternalInput")
    d_WT = nc.dram_tensor("WT", [OUT_DIM, IN_DIM], f32, kind="ExternalInput")
    d_a2 = nc.dram_tensor("a2", [OUT_DIM, 2], f32, kind="ExternalInput")
    d_whT = nc.dram_tensor("whT", [OUT_DIM + 2, NPP], f32, kind="ExternalOutput")

    with tile.TileContext(nc) as tc:
        with tc.tile_pool(name="c1", bufs=1) as cp, \
             tc.tile_pool(name="psw", bufs=1, space="PSUM") as psw, \
             tc.tile_pool(name="ps1", bufs=6, space="PSUM") as psp:
            ht_sb = cp.tile([128, NPP], f32)
            nc.sync.dma_start(out=ht_sb[:], in_=d_hT[:])
            w_sb = cp.tile([IN_DIM, OUT_DIM], f32)
            nc.sync.dma_start(out=w_sb[:], in_=d_W[:])
            wt_sb = cp.tile([OUT_DIM, IN_DIM], f32)
            nc.sync.dma_start(out=wt_sb[:], in_=d_WT[:])
            a_sb = cp.tile([OUT_DIM, 2], f32)
            nc.sync.dma_start(out=a_sb[:], in_=d_a2[:])

            waug = cp.tile([IN_DIM, OUT_DIM + 2], f32)
            nc.vector.tensor_copy(out=waug[:, 0:OUT_DIM], in_=w_sb[:])
            ws_ps = psw.tile([IN_DIM, 2], f32, space="PSUM")
            nc.tensor.matmul(out=ws_ps[:], lhsT=wt_sb[:], rhs=a_sb[:],
                             start=True, stop=True)
            nc.vector.tensor_copy(out=waug[:, OUT_DIM:OUT_DIM + 2], in_=ws_ps[:])

            wh_stage = cp.tile([OUT_DIM + 2, NPP], f32)
            for r in range(RT):
                wh_ps = psp.tile([OUT_DIM + 2, 128], f32, space="PSUM")
                nc.tensor.matmul(out=wh_ps[:], lhsT=waug[:],
                                 rhs=ht_sb[:, r * 128:(r + 1) * 128],
                                 start=True, stop=True)
                nc.vector.tensor_copy(out=wh_stage[:, r * 128:(r + 1) * 128],
                                      in_=wh_ps[:])
            nc.sync.dma_start(out=d_whT[:], in_=wh_stage[:])
    nc.compile()
    return nc


def _build_pass2(Tj, Ttot):
    from concourse import bacc, mybir
    import concourse.tile as tile

    f32 = mybir.dt.float32
    i32 = mybir.dt.int32
    alu = mybir.AluOpType
    act = mybir.ActivationFunctionType
    W65 = OUT_DIM + 1

    base = np.zeros(BPC + 1, np.int64)
    base[1:] = np.cumsum(Tj)
    assert base[-1] == Ttot

    nc = bacc.Bacc("TRN2", target_bir_lowering=False, debug=False)
    d_msg = nc.dram_tensor("msg", [128, Ttot * W65], f32, kind="ExternalInput")
    d_qs = nc.dram_tensor("qs", [128, Ttot], f32, kind="ExternalInput")
    d_qd = nc.dram_tensor("qd", [128, Ttot], f32, kind="ExternalInput")
    d_rr = nc.dram_tensor("rr", [128, Ttot], f32, kind="ExternalInput")
    d_out = nc.dram_tensor("out", [NPP, OUT_DIM], f32, kind="ExternalOutput")

    n_sb = BPC // SBB
    with tile.TileContext(nc) as tc:
        with tc.tile_pool(name="c2", bufs=1) as cp, \
             tc.tile_pool(name="gp", bufs=4) as gp, \
             tc.tile_pool(name="mp", bufs=4) as mp, \
             tc.tile_pool(name="fp", bufs=8) as fp, \
             tc.tile_pool(name="op", bufs=3) as op, \
             tc.tile_pool(name="pp", bufs=8, space="PSUM") as pp:

            iota_i = cp.tile([128, BW], i32)
            nc.gpsimd.iota(iota_i[:], pattern=[[1, BW]], base=0, channel_multiplier=0)
            iota_f = cp.tile([128, BW], f32)
            nc.vector.tensor_copy(out=iota_f[:], in_=iota_i[:])

            qs_sb = cp.tile([128, Ttot], f32)
            nc.sync.dma_start(out=qs_sb[:], in_=d_qs[:])
            qd_sb = cp.tile([128, Ttot], f32)
            nc.sync.dma_start(out=qd_sb[:], in_=d_qd[:])
            rr_sb = cp.tile([128, Ttot], f32)
            nc.sync.dma_start(out=rr_sb[:], in_=d_rr[:])

            # exp(leakyrelu(qs + qd)) for every slot, one shot
            ex_sb = cp.tile([128, Ttot], f32)
            nc.vector.tensor_tensor(out=ex_sb[:], in0=qs_sb[:], in1=qd_sb[:], op=alu.add)
            sc_sb = cp.tile([128, Ttot], f32)
            nc.vector.tensor_scalar(out=sc_sb[:], in0=ex_sb[:], scalar1=NEG_SLOPE,
                                    scalar2=None, op0=alu.mult)
            nc.vector.tensor_tensor(out=ex_sb[:], in0=ex_sb[:], in1=sc_sb[:], op=alu.max)
            nc.scalar.activation(out=ex_sb[:], in_=ex_sb[:], func=act.Exp)

            for s in range(n_sb):
                j0, j1 = s * SBB, (s + 1) * SBB
                t0, t1 = int(base[j0]), int(base[j1])
                T_s = t1 - t0
                G = gp.tile([128, T_s * W65], f32, tag="G")
                nc.sync.dma_start(out=G[:], in_=d_msg[:, t0 * W65:t1 * W65])

                # M[p, (t,j)] = exp[p,t] * (iota_j == rrel[p,t]) — batched
                M = mp.tile([128, T_s * BW], f32, tag="M")
                eng = nc.vector
                eng.tensor_tensor(
                    out=M[:],
                    in0=iota_f[:].rearrange("p (o f) -> p o f", o=1).to_broadcast([128, T_s, BW]),
                    in1=rr_sb[:, t0:t1].rearrange("p (t o) -> p t o", o=1).to_broadcast([128, T_s, BW]),
                    op=alu.is_equal)
                eng.tensor_tensor(
                    out=M[:], in0=M[:],
                    in1=ex_sb[:, t0:t1].rearrange("p (t o) -> p t o", o=1).to_broadcast([128, T_s, BW]),
                    op=alu.mult)

                out_stage = op.tile([BW, SBB * BW], f32, tag="ost")
                for j in range(j0, j1):
                    tj = int(Tj[j])
                    ps = pp.tile([BW, W65], f32, space="PSUM", tag="ps")
                    for t in range(tj):
                        rel = int(base[j]) + t - t0
                        nc.tensor.matmul(out=ps[:],
                                         lhsT=M[:, rel * BW:(rel + 1) * BW],
                                         rhs=G[:, rel * W65:(rel + 1) * W65],
                                         start=(t == 0), stop=(t == tj - 1))
                    dtmp = fp.tile([BW, 1], f32, tag="dt")
                    nc.vector.tensor_scalar(out=dtmp[:], in0=ps[:, OUT_DIM:W65],
                                            scalar1=1e-10, scalar2=None, op0=alu.add)
                    dinv = fp.tile([BW, 1], f32, tag="di")
                    nc.vector.reciprocal(out=dinv[:], in_=dtmp[:])
                    jr = j - j0
                    nc.scalar.activation(out=out_stage[:, jr * BW:(jr + 1) * BW],
                                         in_=ps[:, 0:OUT_DIM], func=act.Copy,
                                         scale=dinv[:])
                out_ap = d_out[j0 * BW:j1 * BW, :].rearrange("(b p) f -> p b f", p=BW)
                in_ap = out_stage[:].rearrange("p (b f) -> p b f", b=SBB)
                nc.sync.dma_start(out=out_ap, in_=in_ap)
    nc.compile()
    return nc


def _prep_structure(row, col):
    """Bucket edges by dest-node 64-block; permute blocks onto (core, slot)
    pairs so that blocks sharing a slot index have similar edge counts
    (shrinks the shared per-slot tile count); assign each edge a slot
    (partition p, tile column t) in its block's 128-edge tiles."""
    NGB = CORES * BPC                       # 1568 block slots (1563 real)
    gb = row // BW                          # global 64-node block per edge
    cnt = np.bincount(gb, minlength=NGB)
    sorted_ids = np.argsort(-cnt, kind="stable")
    blk_core = np.empty(NGB, np.int64)
    blk_slot = np.empty(NGB, np.int64)
    k = np.arange(NGB)
    blk_core[sorted_ids] = k % CORES
    blk_slot[sorted_ids] = k // CORES
    # per slot j: max count over its 8 assigned blocks (sorted -> first of 8)
    Tj = np.maximum(1, (cnt[sorted_ids[::CORES]] + 127) // 128)
    base = np.zeros(BPC + 1, np.int64)
    base[1:] = np.cumsum(Tj)
    Ttot = int(base[-1])

    key = blk_core[gb] * BPC + blk_slot[gb]
    kcnt = np.bincount(key, minlength=NGB)
    order = np.argsort(key, kind="stable")
    starts = np.zeros(NGB, np.int64)
    starts[1:] = np.cumsum(kcnt)[:-1]
    rank = np.arange(N_EDGES, dtype=np.int64) - np.repeat(starts, kcnt)
    key_s = key[order]
    core_s = key_s // BPC
    slot_s = key_s - core_s * BPC
    t_loc = rank >> 7
    p_s = rank & 127
    tglob = base[slot_s] + t_loc
    return dict(order=order, core_s=core_s, p_s=p_s, tglob=tglob,
                gb_s=gb[order], Tj=Tj, base=base, Ttot=Ttot,
                sorted_ids=sorted_ids)


def _run_spmd(nc, in_maps, trace=False):
    from concourse import bass_utils
    res = bass_utils.run_bass_kernel_spmd(
        nc, in_maps, core_ids=list(range(CORES)), trace=trace)
    return res


def kernel(h, row, col, W, a):
    trace = bool(os.environ.get("GAT_TRACE"))
    if trace:
        try:
            import ntff_shim
            ntff_shim.install()
        except Exception:
            trace = False

    h = np.ascontiguousarray(np.asarray(h, dtype=np.float32))
    W = np.ascontiguousarray(np.asarray(W, dtype=np.float32))
    a = np.ascontiguousarray(np.asarray(a, dtype=np.float32)).reshape(2 * OUT_DIM)
    row = np.asarray(row).astype(np.int64)
    col = np.asarray(col).astype(np.int64)

    # ---- pass 1: Wh / s_src / s_dst, node-sharded ----
    nc1 = _build_pass1()
    WT = np.ascontiguousarray(W.T)
    a2 = np.ascontiguousarray(np.stack([a[:OUT_DIM], a[OUT_DIM:]], axis=1))
    in_maps1 = []
    for c in range(CORES):
        hpad = np.zeros((NPP, IN_DIM), np.float32)
        hpad[:NPC] = h[c * NPC:(c + 1) * NPC]
        in_maps1.append({"hT": np.ascontiguousarray(hpad.T), "Wm": W,
                         "WT": WT, "a2": a2})
    res1 = _run_spmd(nc1, in_maps1, trace=trace)
    if trace:
        LAST_STATS["pass1_ns"] = res1.exec_time_ns

    WhA = np.ones((N_NODES, OUT_DIM + 1), np.float32)
    s_src = np.empty(N_NODES, np.float32)
    s_dst = np.empty(N_NODES, np.float32)
    for c in range(CORES):
        whT = res1.results[c]["whT"]
        WhA[c * NPC:(c + 1) * NPC, :OUT_DIM] = whT[:OUT_DIM, :NPC].T
        s_src[c * NPC:(c + 1) * NPC] = whT[OUT_DIM, :NPC]
        s_dst[c * NPC:(c + 1) * NPC] = whT[OUT_DIM + 1, :NPC]

    # ---- host: edge-slot structure + replicated-Wh message streams ----
    st = _prep_structure(row, col)
    Tj, Ttot = st["Tj"], st["Ttot"]
    W65 = OUT_DIM + 1
    cs, ps, tg = st["core_s"], st["p_s"], st["tglob"]
    row_s = row[st["order"]]
    col_s = col[st["order"]]

    msg = np.zeros((CORES, 128, Ttot, W65), np.float32)
    msg[cs, ps, tg] = WhA[col_s]
    qs = np.full((CORES, 128, Ttot), PAD_Q, np.float32)
    qs[cs, ps, tg] = s_src[row_s]
    qd = np.full((CORES, 128, Ttot), PAD_Q, np.float32)
    qd[cs, ps, tg] = s_dst[col_s]
    rr = np.zeros((CORES, 128, Ttot), np.float32)
    rr[cs, ps, tg] = (row_s - st["gb_s"] * BW).astype(np.float32)

    # ---- pass 2: attention + segment sum ----
    nc2 = _build_pass2(Tj, Ttot)
    in_maps2 = [{"msg": msg[c].reshape(128, Ttot * W65), "qs": qs[c],
                 "qd": qd[c], "rr": rr[c]} for c in range(CORES)]
    res2 = _run_spmd(nc2, in_maps2, trace=trace)
    if trace:
        LAST_STATS["pass2_ns"] = res2.exec_time_ns
        LAST_STATS["total_ns"] = (res1.exec_time_ns or 0) + (res2.exec_time_ns or 0)

    out = np.empty((N_NODES, OUT_DIM), np.float32)
    sorted_ids = st["sorted_ids"]
    NGB_REAL = (N_NODES + BW - 1) // BW
    for c in range(CORES):
        dev = res2.results[c]["out"]
        for j in range(BPC):
            g = int(sorted_ids[j * CORES + c])
            if g >= NGB_REAL:
                continue
            sz = min(BW, N_NODES - g * BW)
            out[g * BW:g * BW + sz] = dev[j * BW:j * BW + sz]
    return out


# revision 18
# speedup vs baseline: 2.1990x; 1.0209x over previous
"""GAT influence layer on 8 Trainium2 NeuronCores (Bass/Tile).

Strategy (edge-parallel, row-sharded):
  Pass 1 (device): each core computes its 12.5k-node slice of
      Wh = h @ W, s_src = Wh @ a_src, s_dst = Wh @ a_dst
      via TensorE matmuls against an augmented weight matrix.
  Host: replicates/permutes device-computed Wh into per-core edge-slot
      streams (edges bucketed by destination-node block, 128-edge tiles),
      plus per-slot q_src/q_dst/row_rel arrays.  Data movement only.
  Pass 2 (device): one-shot exp(leakyrelu(q_src+q_dst)); per superblock a
      batched exp-weighted one-hot selection matrix (two DVE tensor_tensor
      ops with broadcast APs); the softmax-weighted segment-sum as
      PSUM-accumulated TensorE matmuls over the message stream; deferred
      division by the per-node denominator (the global max-subtract of the
      reference cancels analytically in the softmax).
  Host: concatenates per-core node-partitioned outputs.
"""

import os
import numpy as np

N_NODES = 100000
N_EDGES = 1600000
IN_DIM = 128
OUT_DIM = 64
NEG_SLOPE = 0.2
CORES = 8
NPC = N_NODES // CORES          # nodes per core (12500)
BW = 64                         # nodes per block (matmul window)
BPC = (NPC + BW - 1) // BW      # blocks per core (196)
NPP = BPC * BW                  # padded nodes per core (12544)
RT = NPP // 128                 # pass-1 row tiles per core (98)
SBB = 7                         # blocks per superblock (196/7 = 28 sbs)
PAD_Q = -30000.0                # pad-slot attention logit -> exp == 0

LAST_STATS = {}


def _build_pass1():
    from concourse import bacc, mybir
    import concourse.tile as tile

    f32 = mybir.dt.float32
    nc = bacc.Bacc("TRN2", target_bir_lowering=False, debug=False)
    d_hT = nc.dram_tensor("hT", [128, NPP], f32, kind="ExternalInput")
    d_W = nc.dram_tensor("Wm", [IN_DIM, OUT_DIM], f32, kind="ExternalInput")
    d_WT = nc.dram_tensor("WT", [OUT_DIM, IN_DIM], f32, kind="ExternalInput")
    d_a2 = nc.dram_tensor("a2", [OUT_DIM, 2], f32, kind="ExternalInput")
    d_whT = nc.dram_tensor("whT", [OUT_DIM + 2, NPP], f32, kind="ExternalOutput")

    NW = 512                    # moving-operand width (fp32 max)
    nck = (NPP + NW - 1) // NW  # 25 chunks (last partial: 256)
    with tile.TileContext(nc) as tc:
        with tc.tile_pool(name="c1", bufs=1) as cp, \
             tc.tile_pool(name="ht1", bufs=4) as hp, \
             tc.tile_pool(name="wo1", bufs=4) as wo, \
             tc.tile_pool(name="psw", bufs=1, space="PSUM") as psw, \
             tc.tile_pool(name="ps1", bufs=6, space="PSUM") as psp:
            w_sb = cp.tile([IN_DIM, OUT_DIM], f32)
            nc.sync.dma_start(out=w_sb[:], in_=d_W[:])
            wt_sb = cp.tile([OUT_DIM, IN_DIM], f32)
            nc.sync.dma_start(out=wt_sb[:], in_=d_WT[:])
            a_sb = cp.tile([OUT_DIM, 2], f32)
            nc.sync.dma_start(out=a_sb[:], in_=d_a2[:])

            waug = cp.tile([IN_DIM, OUT_DIM + 2], f32)
            nc.vector.tensor_copy(out=waug[:, 0:OUT_DIM], in_=w_sb[:])
            ws_ps = psw.tile([IN_DIM, 2], f32, space="PSUM")
            nc.tensor.matmul(out=ws_ps[:], lhsT=wt_sb[:], rhs=a_sb[:],
                             start=True, stop=True)
            nc.vector.tensor_copy(out=waug[:, OUT_DIM:OUT_DIM + 2], in_=ws_ps[:])

            for r in range(nck):
                c0, c1 = r * NW, min((r + 1) * NW, NPP)
                w = c1 - c0
                ht = hp.tile([128, NW], f32, tag="ht")
                nc.sync.dma_start(out=ht[:, :w], in_=d_hT[:, c0:c1])
                wh_ps = psp.tile([OUT_DIM + 2, NW], f32, space="PSUM")
                nc.tensor.matmul(out=wh_ps[:, :w], lhsT=waug[:], rhs=ht[:, :w],
                                 start=True, stop=True)
                wh_sb = wo.tile([OUT_DIM + 2, NW], f32, tag="wh")
                nc.vector.tensor_copy(out=wh_sb[:, :w], in_=wh_ps[:, :w])
                nc.sync.dma_start(out=d_whT[:, c0:c1], in_=wh_sb[:, :w])
    nc.compile()
    return nc


def _build_pass2(Tj, Ttot):
    from concourse import bacc, mybir
    import concourse.tile as tile

    f32 = mybir.dt.float32
    i32 = mybir.dt.int32
    alu = mybir.AluOpType
    act = mybir.ActivationFunctionType
    W65 = OUT_DIM + 1

    base = np.zeros(BPC + 1, np.int64)
    base[1:] = np.cumsum(Tj)
    assert base[-1] == Ttot

    nc = bacc.Bacc("TRN2", target_bir_lowering=False, debug=False)
    d_msg = nc.dram_tensor("msg", [128, Ttot * W65], f32, kind="ExternalInput")
    d_qrr = nc.dram_tensor("qrr", [128, 3 * Ttot], f32, kind="ExternalInput")
    d_out = nc.dram_tensor("out", [NPP, OUT_DIM], f32, kind="ExternalOutput")

    n_sb = BPC // SBB
    with tile.TileContext(nc) as tc:
        with tc.tile_pool(name="c2", bufs=1) as cp, \
             tc.tile_pool(name="gp", bufs=4) as gp, \
             tc.tile_pool(name="mp", bufs=4) as mp, \
             tc.tile_pool(name="fp", bufs=8) as fp, \
             tc.tile_pool(name="op", bufs=3) as op, \
             tc.tile_pool(name="pp", bufs=8, space="PSUM") as pp:

            iota_i = cp.tile([128, BW], i32)
            nc.gpsimd.iota(iota_i[:], pattern=[[1, BW]], base=0, channel_multiplier=0)
            iota_f = cp.tile([128, BW], f32)
            nc.vector.tensor_copy(out=iota_f[:], in_=iota_i[:])

            qrr_sb = cp.tile([128, 3 * Ttot], f32)
            nc.sync.dma_start(out=qrr_sb[:], in_=d_qrr[:])
            qs_sb = qrr_sb[:, 0:Ttot]
            qd_sb = qrr_sb[:, Ttot:2 * Ttot]
            rr_sb = qrr_sb[:, 2 * Ttot:3 * Ttot]

            # exp(leakyrelu(qs + qd)) for every slot, one shot
            ex_sb = cp.tile([128, Ttot], f32)
            nc.vector.tensor_tensor(out=ex_sb[:], in0=qs_sb, in1=qd_sb, op=alu.add)
            sc_sb = cp.tile([128, Ttot], f32)
            nc.vector.tensor_scalar(out=sc_sb[:], in0=ex_sb[:], scalar1=NEG_SLOPE,
                                    scalar2=None, op0=alu.mult)
            nc.vector.tensor_tensor(out=ex_sb[:], in0=ex_sb[:], in1=sc_sb[:], op=alu.max)
            nc.scalar.activation(out=ex_sb[:], in_=ex_sb[:], func=act.Exp)

            for s in range(n_sb):
                j0, j1 = s * SBB, (s + 1) * SBB
                t0, t1 = int(base[j0]), int(base[j1])
                T_s = t1 - t0
                G = gp.tile([128, T_s * W65], f32, tag="G")
                nc.sync.dma_start(out=G[:], in_=d_msg[:, t0 * W65:t1 * W65])

                # M[p, (t,j)] = exp[p,t] * (iota_j == rrel[p,t]) — batched
                M = mp.tile([128, T_s * BW], f32, tag="M")
                eng = nc.vector
                eng.tensor_tensor(
                    out=M[:],
                    in0=iota_f[:].rearrange("p (o f) -> p o f", o=1).to_broadcast([128, T_s, BW]),
                    in1=rr_sb[:, t0:t1].rearrange("p (t o) -> p t o", o=1).to_broadcast([128, T_s, BW]),
                    op=alu.is_equal)
                eng.tensor_tensor(
                    out=M[:], in0=M[:],
                    in1=ex_sb[:, t0:t1].rearrange("p (t o) -> p t o", o=1).to_broadcast([128, T_s, BW]),
                    op=alu.mult)

                out_stage = op.tile([BW, SBB * BW], f32, tag="ost")
                for j in range(j0, j1):
                    tj = int(Tj[j])
                    ps = pp.tile([BW, W65], f32, space="PSUM", tag="ps")
                    for t in range(tj):
                        rel = int(base[j]) + t - t0
                        nc.tensor.matmul(out=ps[:],
                                         lhsT=M[:, rel * BW:(rel + 1) * BW],
                                         rhs=G[:, rel * W65:(rel + 1) * W65],
                                         start=(t == 0), stop=(t == tj - 1))
                    dtmp = fp.tile([BW, 1], f32, tag="dt")
                    nc.vector.tensor_scalar(out=dtmp[:], in0=ps[:, OUT_DIM:W65],
                                            scalar1=1e-10, scalar2=None, op0=alu.add)
                    dinv = fp.tile([BW, 1], f32, tag="di")
                    nc.vector.reciprocal(out=dinv[:], in_=dtmp[:])
                    jr = j - j0
                    nc.scalar.activation(out=out_stage[:, jr * BW:(jr + 1) * BW],
                                         in_=ps[:, 0:OUT_DIM], func=act.Copy,
                                         scale=dinv[:])
                out_ap = d_out[j0 * BW:j1 * BW, :].rearrange("(b p) f -> p b f", p=BW)
                in_ap = out_stage[:].rearrange("p (b f) -> p b f", b=SBB)
                nc.sync.dma_start(out=out_ap, in_=in_ap)
    nc.compile()
    return nc


def _prep_structure(row, col):
    """Bucket edges by dest-node 64-block; permute blocks onto (core, slot)
    pairs so that blocks sharing a slot index have similar edge counts
    (shrinks the shared per-slot tile count); assign each edge a slot
    (partition p, tile column t) in its block's 128-edge tiles."""
    NGB = CORES * BPC                       # 1568 block slots (1563 real)
    gb = row // BW                          # global 64-node block per edge
    cnt = np.bincount(gb, minlength=NGB)
    sorted_ids = np.argsort(-cnt, kind="stable")
    blk_core = np.empty(NGB, np.int64)
    blk_slot = np.empty(NGB, np.int64)
    k = np.arange(NGB)
    blk_core[sorted_ids] = k % CORES
    blk_slot[sorted_ids] = k // CORES
    # per slot j: max count over its 8 assigned blocks (sorted -> first of 8)
    Tj = np.maximum(1, (cnt[sorted_ids[::CORES]] + 127) // 128)
    base = np.zeros(BPC + 1, np.int64)
    base[1:] = np.cumsum(Tj)
    Ttot = int(base[-1])

    key = blk_core[gb] * BPC + blk_slot[gb]
    kcnt = np.bincount(key, minlength=NGB)
    order = np.argsort(key, kind="stable")
    starts = np.zeros(NGB, np.int64)
    starts[1:] = np.cumsum(kcnt)[:-1]
    rank = np.arange(N_EDGES, dtype=np.int64) - np.repeat(starts, kcnt)
    key_s = key[order]
    core_s = key_s // BPC
    slot_s = key_s - core_s * BPC
    t_loc = rank >> 7
    p_s = rank & 127
    tglob = base[slot_s] + t_loc
    return dict(order=order, core_s=core_s, p_s=p_s, tglob=tglob,
                gb_s=gb[order], Tj=Tj, base=base, Ttot=Ttot,
                sorted_ids=sorted_ids)


def _run_spmd(nc, in_maps, trace=False):
    from concourse import bass_utils
    res = bass_utils.run_bass_kernel_spmd(
        nc, in_maps, core_ids=list(range(CORES)), trace=trace)
    return res


def kernel(h, row, col, W, a):
    trace = bool(os.environ.get("GAT_TRACE"))
    if trace:
        try:
            import ntff_shim
            ntff_shim.install()
        except Exception:
            trace = False

    h = np.ascontiguousarray(np.asarray(h, dtype=np.float32))
    W = np.ascontiguousarray(np.asarray(W, dtype=np.float32))
    a = np.ascontiguousarray(np.asarray(a, dtype=np.float32)).reshape(2 * OUT_DIM)
    row = np.asarray(row).astype(np.int64)
    col = np.asarray(col).astype(np.int64)

    # ---- pass 1: Wh / s_src / s_dst, node-sharded ----
    nc1 = _build_pass1()
    WT = np.ascontiguousarray(W.T)
    a2 = np.ascontiguousarray(np.stack([a[:OUT_DIM], a[OUT_DIM:]], axis=1))
    in_maps1 = []
    for c in range(CORES):
        hpad = np.zeros((NPP, IN_DIM), np.float32)
        hpad[:NPC] = h[c * NPC:(c + 1) * NPC]
        in_maps1.append({"hT": np.ascontiguousarray(hpad.T), "Wm": W,
                         "WT": WT, "a2": a2})
    res1 = _run_spmd(nc1, in_maps1, trace=trace)
    if trace:
        LAST_STATS["pass1_ns"] = res1.exec_time_ns

    WhA = np.ones((N_NODES, OUT_DIM + 1), np.float32)
    s_src = np.empty(N_NODES, np.float32)
    s_dst = np.empty(N_NODES, np.float32)
    for c in range(CORES):
        whT = res1.results[c]["whT"]
        WhA[c * NPC:(c + 1) * NPC, :OUT_DIM] = whT[:OUT_DIM, :NPC].T
        s_src[c * NPC:(c + 1) * NPC] = whT[OUT_DIM, :NPC]
        s_dst[c * NPC:(c + 1) * NPC] = whT[OUT_DIM + 1, :NPC]

    # ---- host: edge-slot structure + replicated-Wh message streams ----
    st = _prep_structure(row, col)
    Tj, Ttot = st["Tj"], st["Ttot"]
    W65 = OUT_DIM + 1
    cs, ps, tg = st["core_s"], st["p_s"], st["tglob"]
    row_s = row[st["order"]]
    col_s = col[st["order"]]

    msg = np.zeros((CORES, 128, Ttot, W65), np.float32)
    msg[cs, ps, tg] = WhA[col_s]
    qs = np.full((CORES, 128, Ttot), PAD_Q, np.float32)
    qs[cs, ps, tg] = s_src[row_s]
    qd = np.full((CORES, 128, Ttot), PAD_Q, np.float32)
    qd[cs, ps, tg] = s_dst[col_s]
    rr = np.zeros((CORES, 128, Ttot), np.float32)
    rr[cs, ps, tg] = (row_s - st["gb_s"] * BW).astype(np.float32)

    # ---- pass 2: attention + segment sum ----
    nc2 = _build_pass2(Tj, Ttot)
    in_maps2 = [{"msg": msg[c].reshape(128, Ttot * W65),
                 "qrr": np.concatenate([qs[c], qd[c], rr[c]], axis=1)}
                for c in range(CORES)]
    res2 = _run_spmd(nc2, in_maps2, trace=trace)
    if trace:
        LAST_STATS["pass2_ns"] = res2.exec_time_ns
        LAST_STATS["total_ns"] = (res1.exec_time_ns or 0) + (res2.exec_time_ns or 0)

    out = np.empty((N_NODES, OUT_DIM), np.float32)
    sorted_ids = st["sorted_ids"]
    NGB_REAL = (N_NODES + BW - 1) // BW
    for c in range(CORES):
        dev = res2.results[c]["out"]
        for j in range(BPC):
            g = int(sorted_ids[j * CORES + c])
            if g >= NGB_REAL:
                continue
            sz = min(BW, N_NODES - g * BW)
            out[g * BW:g * BW + sz] = dev[j * BW:j * BW + sz]
    return out


# revision 20
# speedup vs baseline: 2.3221x; 1.0560x over previous
"""GAT influence layer on 8 Trainium2 NeuronCores (Bass/Tile).

Strategy (edge-parallel, row-sharded):
  Pass 1 (device): each core computes its 12.5k-node slice of
      Wh = h @ W, s_src = Wh @ a_src, s_dst = Wh @ a_dst
      via TensorE matmuls against an augmented weight matrix.
  Host: replicates/permutes device-computed Wh into per-core edge-slot
      streams (edges bucketed by destination-node block, 128-edge tiles),
      plus per-slot q_src/q_dst/row_rel arrays.  Data movement only.
  Pass 2 (device): one-shot exp(leakyrelu(q_src+q_dst)); per superblock a
      batched exp-weighted one-hot selection matrix (two DVE tensor_tensor
      ops with broadcast APs); the softmax-weighted segment-sum as
      PSUM-accumulated TensorE matmuls over the message stream; deferred
      division by the per-node denominator (the global max-subtract of the
      reference cancels analytically in the softmax).
  Host: concatenates per-core node-partitioned outputs.
"""

import os
import numpy as np

N_NODES = 100000
N_EDGES = 1600000
IN_DIM = 128
OUT_DIM = 64
NEG_SLOPE = 0.2
CORES = 8
NPC = N_NODES // CORES          # nodes per core (12500)
BW = 48                         # nodes per block (matmul window)
BPC = 261                       # block slots per core (8*261 >= ceil(N/48))
NPP2 = BPC * BW                 # padded nodes per core, pass 2 (12528)
NPP = 12544                     # padded nodes per core, pass 1 (98*128)
RT = NPP // 128                 # pass-1 row tiles per core (98)
SBB = 9                         # blocks per superblock (261/9 = 29 sbs)
PAD_Q = -30000.0                # pad-slot attention logit -> exp == 0

LAST_STATS = {}


def _build_pass1():
    from concourse import bacc, mybir
    import concourse.tile as tile

    f32 = mybir.dt.float32
    nc = bacc.Bacc("TRN2", target_bir_lowering=False, debug=False)
    d_hT = nc.dram_tensor("hT", [128, NPP], f32, kind="ExternalInput")
    d_W = nc.dram_tensor("Wm", [IN_DIM, OUT_DIM], f32, kind="ExternalInput")
    d_WT = nc.dram_tensor("WT", [OUT_DIM, IN_DIM], f32, kind="ExternalInput")
    d_a2 = nc.dram_tensor("a2", [OUT_DIM, 2], f32, kind="ExternalInput")
    d_whT = nc.dram_tensor("whT", [OUT_DIM + 2, NPP], f32, kind="ExternalOutput")

    NW = 512                    # moving-operand width (fp32 max)
    nck = (NPP + NW - 1) // NW  # 25 chunks (last partial: 256)
    with tile.TileContext(nc) as tc:
        with tc.tile_pool(name="c1", bufs=1) as cp, \
             tc.tile_pool(name="ht1", bufs=4) as hp, \
             tc.tile_pool(name="wo1", bufs=4) as wo, \
             tc.tile_pool(name="psw", bufs=1, space="PSUM") as psw, \
             tc.tile_pool(name="ps1", bufs=6, space="PSUM") as psp:
            w_sb = cp.tile([IN_DIM, OUT_DIM], f32)
            nc.sync.dma_start(out=w_sb[:], in_=d_W[:])
            wt_sb = cp.tile([OUT_DIM, IN_DIM], f32)
            nc.sync.dma_start(out=wt_sb[:], in_=d_WT[:])
            a_sb = cp.tile([OUT_DIM, 2], f32)
            nc.sync.dma_start(out=a_sb[:], in_=d_a2[:])

            waug = cp.tile([IN_DIM, OUT_DIM + 2], f32)
            nc.vector.tensor_copy(out=waug[:, 0:OUT_DIM], in_=w_sb[:])
            ws_ps = psw.tile([IN_DIM, 2], f32, space="PSUM")
            nc.tensor.matmul(out=ws_ps[:], lhsT=wt_sb[:], rhs=a_sb[:],
                             start=True, stop=True)
            nc.vector.tensor_copy(out=waug[:, OUT_DIM:OUT_DIM + 2], in_=ws_ps[:])

            for r in range(nck):
                c0, c1 = r * NW, min((r + 1) * NW, NPP)
                w = c1 - c0
                ht = hp.tile([128, NW], f32, tag="ht")
                nc.sync.dma_start(out=ht[:, :w], in_=d_hT[:, c0:c1])
                wh_ps = psp.tile([OUT_DIM + 2, NW], f32, space="PSUM")
                nc.tensor.matmul(out=wh_ps[:, :w], lhsT=waug[:], rhs=ht[:, :w],
                                 start=True, stop=True)
                wh_sb = wo.tile([OUT_DIM + 2, NW], f32, tag="wh")
                nc.vector.tensor_copy(out=wh_sb[:, :w], in_=wh_ps[:, :w])
                nc.sync.dma_start(out=d_whT[:, c0:c1], in_=wh_sb[:, :w])
    nc.compile()
    return nc


def _build_pass2(Tj, Ttot):
    from concourse import bacc, mybir
    import concourse.tile as tile

    f32 = mybir.dt.float32
    i32 = mybir.dt.int32
    alu = mybir.AluOpType
    act = mybir.ActivationFunctionType
    W65 = OUT_DIM + 1

    base = np.zeros(BPC + 1, np.int64)
    base[1:] = np.cumsum(Tj)
    assert base[-1] == Ttot

    nc = bacc.Bacc("TRN2", target_bir_lowering=False, debug=False)
    d_msg = nc.dram_tensor("msg", [128, Ttot * W65], f32, kind="ExternalInput")
    d_qrr = nc.dram_tensor("qrr", [128, 3 * Ttot], f32, kind="ExternalInput")
    d_out = nc.dram_tensor("out", [NPP2, OUT_DIM], f32, kind="ExternalOutput")

    n_sb = BPC // SBB
    with tile.TileContext(nc) as tc:
        with tc.tile_pool(name="c2", bufs=1) as cp, \
             tc.tile_pool(name="gp", bufs=4) as gp, \
             tc.tile_pool(name="mp", bufs=4) as mp, \
             tc.tile_pool(name="fp", bufs=8) as fp, \
             tc.tile_pool(name="op", bufs=3) as op, \
             tc.tile_pool(name="pp", bufs=8, space="PSUM") as pp:

            iota_i = cp.tile([128, BW], i32)
            nc.gpsimd.iota(iota_i[:], pattern=[[1, BW]], base=0, channel_multiplier=0)
            iota_f = cp.tile([128, BW], f32)
            nc.vector.tensor_copy(out=iota_f[:], in_=iota_i[:])

            qrr_sb = cp.tile([128, 3 * Ttot], f32)
            nc.sync.dma_start(out=qrr_sb[:], in_=d_qrr[:])
            qs_sb = qrr_sb[:, 0:Ttot]
            qd_sb = qrr_sb[:, Ttot:2 * Ttot]
            rr_sb = qrr_sb[:, 2 * Ttot:3 * Ttot]

            # exp(leakyrelu(qs + qd)) for every slot, one shot
            ex_sb = cp.tile([128, Ttot], f32)
            nc.vector.tensor_tensor(out=ex_sb[:], in0=qs_sb, in1=qd_sb, op=alu.add)
            sc_sb = cp.tile([128, Ttot], f32)
            nc.vector.tensor_scalar(out=sc_sb[:], in0=ex_sb[:], scalar1=NEG_SLOPE,
                                    scalar2=None, op0=alu.mult)
            nc.vector.tensor_tensor(out=ex_sb[:], in0=ex_sb[:], in1=sc_sb[:], op=alu.max)
            nc.scalar.activation(out=ex_sb[:], in_=ex_sb[:], func=act.Exp)

            for s in range(n_sb):
                j0, j1 = s * SBB, (s + 1) * SBB
                t0, t1 = int(base[j0]), int(base[j1])
                T_s = t1 - t0
                G = gp.tile([128, T_s * W65], f32, tag="G")
                nc.sync.dma_start(out=G[:], in_=d_msg[:, t0 * W65:t1 * W65])

                # M[p, (t,j)] = exp[p,t] * (iota_j == rrel[p,t]) — batched
                M = mp.tile([128, T_s * BW], f32, tag="M")
                eng = nc.vector
                eng.tensor_tensor(
                    out=M[:],
                    in0=iota_f[:].rearrange("p (o f) -> p o f", o=1).to_broadcast([128, T_s, BW]),
                    in1=rr_sb[:, t0:t1].rearrange("p (t o) -> p t o", o=1).to_broadcast([128, T_s, BW]),
                    op=alu.is_equal)
                eng.tensor_tensor(
                    out=M[:], in0=M[:],
                    in1=ex_sb[:, t0:t1].rearrange("p (t o) -> p t o", o=1).to_broadcast([128, T_s, BW]),
                    op=alu.mult)

                out_stage = op.tile([BW, SBB * OUT_DIM], f32, tag="ost")
                for j in range(j0, j1):
                    tj = int(Tj[j])
                    ps = pp.tile([BW, W65], f32, space="PSUM", tag="ps")
                    for t in range(tj):
                        rel = int(base[j]) + t - t0
                        nc.tensor.matmul(out=ps[:],
                                         lhsT=M[:, rel * BW:(rel + 1) * BW],
                                         rhs=G[:, rel * W65:(rel + 1) * W65],
                                         start=(t == 0), stop=(t == tj - 1))
                    dtmp = fp.tile([BW, 1], f32, tag="dt")
                    nc.vector.tensor_scalar(out=dtmp[:], in0=ps[:, OUT_DIM:W65],
                                            scalar1=1e-10, scalar2=None, op0=alu.add)
                    dinv = fp.tile([BW, 1], f32, tag="di")
                    nc.vector.reciprocal(out=dinv[:], in_=dtmp[:])
                    jr = j - j0
                    nc.scalar.activation(out=out_stage[:, jr * OUT_DIM:(jr + 1) * OUT_DIM],
                                         in_=ps[:, 0:OUT_DIM], func=act.Copy,
                                         scale=dinv[:])
                out_ap = d_out[j0 * BW:j1 * BW, :].rearrange("(b p) f -> p b f", p=BW)
                in_ap = out_stage[:].rearrange("p (b f) -> p b f", b=SBB)
                nc.sync.dma_start(out=out_ap, in_=in_ap)
    nc.compile()
    return nc


def _prep_structure(row, col):
    """Bucket edges by dest-node 64-block; permute blocks onto (core, slot)
    pairs so that blocks sharing a slot index have similar edge counts
    (shrinks the shared per-slot tile count); assign each edge a slot
    (partition p, tile column t) in its block's 128-edge tiles."""
    NGB = CORES * BPC                       # 1568 block slots (1563 real)
    gb = row // BW                          # global 64-node block per edge
    cnt = np.bincount(gb, minlength=NGB)
    sorted_ids = np.argsort(-cnt, kind="stable")
    blk_core = np.empty(NGB, np.int64)
    blk_slot = np.empty(NGB, np.int64)
    k = np.arange(NGB)
    blk_core[sorted_ids] = k % CORES
    blk_slot[sorted_ids] = k // CORES
    # per slot j: max count over its 8 assigned blocks (sorted -> first of 8)
    Tj = np.maximum(1, (cnt[sorted_ids[::CORES]] + 127) // 128)
    base = np.zeros(BPC + 1, np.int64)
    base[1:] = np.cumsum(Tj)
    Ttot = int(base[-1])

    key = blk_core[gb] * BPC + blk_slot[gb]
    kcnt = np.bincount(key, minlength=NGB)
    order = np.argsort(key, kind="stable")
    starts = np.zeros(NGB, np.int64)
    starts[1:] = np.cumsum(kcnt)[:-1]
    rank = np.arange(N_EDGES, dtype=np.int64) - np.repeat(starts, kcnt)
    key_s = key[order]
    core_s = key_s // BPC
    slot_s = key_s - core_s * BPC
    t_loc = rank >> 7
    p_s = rank & 127
    tglob = base[slot_s] + t_loc
    return dict(order=order, core_s=core_s, p_s=p_s, tglob=tglob,
                gb_s=gb[order], Tj=Tj, base=base, Ttot=Ttot,
                sorted_ids=sorted_ids)


def _run_spmd(nc, in_maps, trace=False):
    from concourse import bass_utils
    res = bass_utils.run_bass_kernel_spmd(
        nc, in_maps, core_ids=list(range(CORES)), trace=trace)
    return res


def kernel(h, row, col, W, a):
    trace = bool(os.environ.get("GAT_TRACE"))
    if trace:
        try:
            import ntff_shim
            ntff_shim.install()
        except Exception:
            trace = False

    h = np.ascontiguousarray(np.asarray(h, dtype=np.float32))
    W = np.ascontiguousarray(np.asarray(W, dtype=np.float32))
    a = np.ascontiguousarray(np.asarray(a, dtype=np.float32)).reshape(2 * OUT_DIM)
    row = np.asarray(row).astype(np.int64)
    col = np.asarray(col).astype(np.int64)

    # ---- pass 1: Wh / s_src / s_dst, node-sharded ----
    nc1 = _build_pass1()
    WT = np.ascontiguousarray(W.T)
    a2 = np.ascontiguousarray(np.stack([a[:OUT_DIM], a[OUT_DIM:]], axis=1))
    in_maps1 = []
    for c in range(CORES):
        hpad = np.zeros((NPP, IN_DIM), np.float32)
        hpad[:NPC] = h[c * NPC:(c + 1) * NPC]
        in_maps1.append({"hT": np.ascontiguousarray(hpad.T), "Wm": W,
                         "WT": WT, "a2": a2})
    res1 = _run_spmd(nc1, in_maps1, trace=trace)
    if trace:
        LAST_STATS["pass1_ns"] = res1.exec_time_ns

    WhA = np.ones((N_NODES, OUT_DIM + 1), np.float32)
    s_src = np.empty(N_NODES, np.float32)
    s_dst = np.empty(N_NODES, np.float32)
    for c in range(CORES):
        whT = res1.results[c]["whT"]
        WhA[c * NPC:(c + 1) * NPC, :OUT_DIM] = whT[:OUT_DIM, :NPC].T
        s_src[c * NPC:(c + 1) * NPC] = whT[OUT_DIM, :NPC]
        s_dst[c * NPC:(c + 1) * NPC] = whT[OUT_DIM + 1, :NPC]

    # ---- host: edge-slot structure + replicated-Wh message streams ----
    st = _prep_structure(row, col)
    Tj, Ttot = st["Tj"], st["Ttot"]
    W65 = OUT_DIM + 1
    cs, ps, tg = st["core_s"], st["p_s"], st["tglob"]
    row_s = row[st["order"]]
    col_s = col[st["order"]]

    msg = np.zeros((CORES, 128, Ttot, W65), np.float32)
    msg[cs, ps, tg] = WhA[col_s]
    qs = np.full((CORES, 128, Ttot), PAD_Q, np.float32)
    qs[cs, ps, tg] = s_src[row_s]
    qd = np.full((CORES, 128, Ttot), PAD_Q, np.float32)
    qd[cs, ps, tg] = s_dst[col_s]
    rr = np.zeros((CORES, 128, Ttot), np.float32)
    rr[cs, ps, tg] = (row_s - st["gb_s"] * BW).astype(np.float32)

    # ---- pass 2: attention + segment sum ----
    nc2 = _build_pass2(Tj, Ttot)
    in_maps2 = [{"msg": msg[c].reshape(128, Ttot * W65),
                 "qrr": np.concatenate([qs[c], qd[c], rr[c]], axis=1)}
                for c in range(CORES)]
    res2 = _run_spmd(nc2, in_maps2, trace=trace)
    if trace:
        LAST_STATS["pass2_ns"] = res2.exec_time_ns
        LAST_STATS["total_ns"] = (res1.exec_time_ns or 0) + (res2.exec_time_ns or 0)

    out = np.empty((N_NODES, OUT_DIM), np.float32)
    sorted_ids = st["sorted_ids"]
    NGB_REAL = (N_NODES + BW - 1) // BW
    for c in range(CORES):
        dev = res2.results[c]["out"]
        for j in range(BPC):
            g = int(sorted_ids[j * CORES + c])
            if g >= NGB_REAL:
                continue
            sz = min(BW, N_NODES - g * BW)
            out[g * BW:g * BW + sz] = dev[j * BW:j * BW + sz]
    return out


# revision 21
# speedup vs baseline: 2.3263x; 1.0018x over previous
"""GAT influence layer on 8 Trainium2 NeuronCores (Bass/Tile).

Strategy (edge-parallel, row-sharded):
  Pass 1 (device): each core computes its 12.5k-node slice of
      Wh = h @ W, s_src = Wh @ a_src, s_dst = Wh @ a_dst
      via TensorE matmuls against an augmented weight matrix.
  Host: replicates/permutes device-computed Wh into per-core edge-slot
      streams (edges bucketed by destination-node block, 128-edge tiles),
      plus per-slot q_src/q_dst/row_rel arrays.  Data movement only.
  Pass 2 (device): one-shot exp(leakyrelu(q_src+q_dst)); per superblock a
      batched exp-weighted one-hot selection matrix (two DVE tensor_tensor
      ops with broadcast APs); the softmax-weighted segment-sum as
      PSUM-accumulated TensorE matmuls over the message stream; deferred
      division by the per-node denominator (the global max-subtract of the
      reference cancels analytically in the softmax).
  Host: concatenates per-core node-partitioned outputs.
"""

import os
import numpy as np

N_NODES = 100000
N_EDGES = 1600000
IN_DIM = 128
OUT_DIM = 64
NEG_SLOPE = 0.2
CORES = 8
NPC = N_NODES // CORES          # nodes per core (12500)
BW = 48                         # nodes per block (matmul window)
BPC = 261                       # block slots per core (8*261 >= ceil(N/48))
NPP2 = BPC * BW                 # padded nodes per core, pass 2 (12528)
NPP = 12544                     # padded nodes per core, pass 1 (98*128)
RT = NPP // 128                 # pass-1 row tiles per core (98)
SBB = 9                         # blocks per superblock (261/9 = 29 sbs)
PAD_Q = -30000.0                # pad-slot attention logit -> exp == 0

LAST_STATS = {}


def _build_pass1():
    from concourse import bacc, mybir
    import concourse.tile as tile

    f32 = mybir.dt.float32
    nc = bacc.Bacc("TRN2", target_bir_lowering=False, debug=False)
    d_hT = nc.dram_tensor("hT", [128, NPP], f32, kind="ExternalInput")
    d_W = nc.dram_tensor("Wm", [IN_DIM, OUT_DIM], f32, kind="ExternalInput")
    d_WT = nc.dram_tensor("WT", [OUT_DIM, IN_DIM], f32, kind="ExternalInput")
    d_a2 = nc.dram_tensor("a2", [OUT_DIM, 2], f32, kind="ExternalInput")
    d_whT = nc.dram_tensor("whT", [OUT_DIM + 2, NPP], f32, kind="ExternalOutput")

    NW = 512                    # moving-operand width (fp32 max)
    nck = (NPP + NW - 1) // NW  # 25 chunks (last partial: 256)
    with tile.TileContext(nc) as tc:
        with tc.tile_pool(name="c1", bufs=1) as cp, \
             tc.tile_pool(name="ht1", bufs=4) as hp, \
             tc.tile_pool(name="wo1", bufs=4) as wo, \
             tc.tile_pool(name="psw", bufs=1, space="PSUM") as psw, \
             tc.tile_pool(name="ps1", bufs=6, space="PSUM") as psp:
            w_sb = cp.tile([IN_DIM, OUT_DIM], f32)
            nc.sync.dma_start(out=w_sb[:], in_=d_W[:])
            wt_sb = cp.tile([OUT_DIM, IN_DIM], f32)
            nc.sync.dma_start(out=wt_sb[:], in_=d_WT[:])
            a_sb = cp.tile([OUT_DIM, 2], f32)
            nc.sync.dma_start(out=a_sb[:], in_=d_a2[:])

            waug = cp.tile([IN_DIM, OUT_DIM + 2], f32)
            nc.vector.tensor_copy(out=waug[:, 0:OUT_DIM], in_=w_sb[:])
            ws_ps = psw.tile([IN_DIM, 2], f32, space="PSUM")
            nc.tensor.matmul(out=ws_ps[:], lhsT=wt_sb[:], rhs=a_sb[:],
                             start=True, stop=True)
            nc.vector.tensor_copy(out=waug[:, OUT_DIM:OUT_DIM + 2], in_=ws_ps[:])

            for r in range(nck):
                c0, c1 = r * NW, min((r + 1) * NW, NPP)
                w = c1 - c0
                ht = hp.tile([128, NW], f32, tag="ht")
                nc.sync.dma_start(out=ht[:, :w], in_=d_hT[:, c0:c1])
                wh_ps = psp.tile([OUT_DIM + 2, NW], f32, space="PSUM")
                nc.tensor.matmul(out=wh_ps[:, :w], lhsT=waug[:], rhs=ht[:, :w],
                                 start=True, stop=True)
                wh_sb = wo.tile([OUT_DIM + 2, NW], f32, tag="wh")
                nc.vector.tensor_copy(out=wh_sb[:, :w], in_=wh_ps[:, :w])
                nc.sync.dma_start(out=d_whT[:, c0:c1], in_=wh_sb[:, :w])
    nc.compile()
    return nc


def _build_pass2(Tj, Ttot, eps_free=False):
    from concourse import bacc, mybir
    import concourse.tile as tile

    f32 = mybir.dt.float32
    i32 = mybir.dt.int32
    alu = mybir.AluOpType
    act = mybir.ActivationFunctionType
    W65 = OUT_DIM + 1

    base = np.zeros(BPC + 1, np.int64)
    base[1:] = np.cumsum(Tj)
    assert base[-1] == Ttot

    nc = bacc.Bacc("TRN2", target_bir_lowering=False, debug=False)
    d_msg = nc.dram_tensor("msg", [128, Ttot * W65], f32, kind="ExternalInput")
    d_qrr = nc.dram_tensor("qrr", [128, 3 * Ttot], f32, kind="ExternalInput")
    d_out = nc.dram_tensor("out", [NPP2, OUT_DIM], f32, kind="ExternalOutput")

    n_sb = BPC // SBB
    with tile.TileContext(nc) as tc:
        with tc.tile_pool(name="c2", bufs=1) as cp, \
             tc.tile_pool(name="gp", bufs=4) as gp, \
             tc.tile_pool(name="mp", bufs=4) as mp, \
             tc.tile_pool(name="fp", bufs=8) as fp, \
             tc.tile_pool(name="op", bufs=3) as op, \
             tc.tile_pool(name="pp", bufs=8, space="PSUM") as pp:

            iota_i = cp.tile([128, BW], i32)
            nc.gpsimd.iota(iota_i[:], pattern=[[1, BW]], base=0, channel_multiplier=0)
            iota_f = cp.tile([128, BW], f32)
            nc.vector.tensor_copy(out=iota_f[:], in_=iota_i[:])

            qrr_sb = cp.tile([128, 3 * Ttot], f32)
            nc.sync.dma_start(out=qrr_sb[:], in_=d_qrr[:])
            qs_sb = qrr_sb[:, 0:Ttot]
            qd_sb = qrr_sb[:, Ttot:2 * Ttot]
            rr_sb = qrr_sb[:, 2 * Ttot:3 * Ttot]

            # exp(leakyrelu(qs + qd)) for every slot, one shot
            ex_sb = cp.tile([128, Ttot], f32)
            nc.vector.tensor_tensor(out=ex_sb[:], in0=qs_sb, in1=qd_sb, op=alu.add)
            sc_sb = cp.tile([128, Ttot], f32)
            nc.vector.tensor_scalar(out=sc_sb[:], in0=ex_sb[:], scalar1=NEG_SLOPE,
                                    scalar2=None, op0=alu.mult)
            nc.vector.tensor_tensor(out=ex_sb[:], in0=ex_sb[:], in1=sc_sb[:], op=alu.max)
            nc.scalar.activation(out=ex_sb[:], in_=ex_sb[:], func=act.Exp)

            for s in range(n_sb):
                j0, j1 = s * SBB, (s + 1) * SBB
                t0, t1 = int(base[j0]), int(base[j1])
                T_s = t1 - t0
                G = gp.tile([128, T_s * W65], f32, tag="G")
                nc.sync.dma_start(out=G[:], in_=d_msg[:, t0 * W65:t1 * W65])

                # M[p, (t,j)] = exp[p,t] * (iota_j == rrel[p,t]) — batched
                M = mp.tile([128, T_s * BW], f32, tag="M")
                eng = nc.vector
                eng.tensor_tensor(
                    out=M[:],
                    in0=iota_f[:].rearrange("p (o f) -> p o f", o=1).to_broadcast([128, T_s, BW]),
                    in1=rr_sb[:, t0:t1].rearrange("p (t o) -> p t o", o=1).to_broadcast([128, T_s, BW]),
                    op=alu.is_equal)
                eng.tensor_tensor(
                    out=M[:], in0=M[:],
                    in1=ex_sb[:, t0:t1].rearrange("p (t o) -> p t o", o=1).to_broadcast([128, T_s, BW]),
                    op=alu.mult)

                out_stage = op.tile([BW, SBB * OUT_DIM], f32, tag="ost")
                for j in range(j0, j1):
                    tj = int(Tj[j])
                    ps = pp.tile([BW, W65], f32, space="PSUM", tag="ps")
                    for t in range(tj):
                        rel = int(base[j]) + t - t0
                        nc.tensor.matmul(out=ps[:],
                                         lhsT=M[:, rel * BW:(rel + 1) * BW],
                                         rhs=G[:, rel * W65:(rel + 1) * W65],
                                         start=(t == 0), stop=(t == tj - 1))
                    dinv = fp.tile([BW, 1], f32, tag="di")
                    if eps_free:
                        nc.vector.reciprocal(out=dinv[:], in_=ps[:, OUT_DIM:W65])
                    else:
                        dtmp = fp.tile([BW, 1], f32, tag="dt")
                        nc.vector.tensor_scalar(out=dtmp[:], in0=ps[:, OUT_DIM:W65],
                                                scalar1=1e-10, scalar2=None, op0=alu.add)
                        nc.vector.reciprocal(out=dinv[:], in_=dtmp[:])
                    jr = j - j0
                    nc.scalar.activation(out=out_stage[:, jr * OUT_DIM:(jr + 1) * OUT_DIM],
                                         in_=ps[:, 0:OUT_DIM], func=act.Copy,
                                         scale=dinv[:])
                out_ap = d_out[j0 * BW:j1 * BW, :].rearrange("(b p) f -> p b f", p=BW)
                in_ap = out_stage[:].rearrange("p (b f) -> p b f", b=SBB)
                nc.sync.dma_start(out=out_ap, in_=in_ap)
    nc.compile()
    return nc


def _prep_structure(row, col):
    """Bucket edges by dest-node 64-block; permute blocks onto (core, slot)
    pairs so that blocks sharing a slot index have similar edge counts
    (shrinks the shared per-slot tile count); assign each edge a slot
    (partition p, tile column t) in its block's 128-edge tiles."""
    NGB = CORES * BPC                       # 1568 block slots (1563 real)
    gb = row // BW                          # global 64-node block per edge
    cnt = np.bincount(gb, minlength=NGB)
    sorted_ids = np.argsort(-cnt, kind="stable")
    blk_core = np.empty(NGB, np.int64)
    blk_slot = np.empty(NGB, np.int64)
    k = np.arange(NGB)
    blk_core[sorted_ids] = k % CORES
    blk_slot[sorted_ids] = k // CORES
    # per slot j: max count over its 8 assigned blocks (sorted -> first of 8)
    Tj = np.maximum(1, (cnt[sorted_ids[::CORES]] + 127) // 128)
    base = np.zeros(BPC + 1, np.int64)
    base[1:] = np.cumsum(Tj)
    Ttot = int(base[-1])

    key = blk_core[gb] * BPC + blk_slot[gb]
    kcnt = np.bincount(key, minlength=NGB)
    order = np.argsort(key, kind="stable")
    starts = np.zeros(NGB, np.int64)
    starts[1:] = np.cumsum(kcnt)[:-1]
    rank = np.arange(N_EDGES, dtype=np.int64) - np.repeat(starts, kcnt)
    key_s = key[order]
    core_s = key_s // BPC
    slot_s = key_s - core_s * BPC
    t_loc = rank >> 7
    p_s = rank & 127
    tglob = base[slot_s] + t_loc
    return dict(order=order, core_s=core_s, p_s=p_s, tglob=tglob,
                gb_s=gb[order], Tj=Tj, base=base, Ttot=Ttot,
                sorted_ids=sorted_ids)


def _run_spmd(nc, in_maps, trace=False):
    from concourse import bass_utils
    res = bass_utils.run_bass_kernel_spmd(
        nc, in_maps, core_ids=list(range(CORES)), trace=trace)
    return res


def kernel(h, row, col, W, a):
    trace = bool(os.environ.get("GAT_TRACE"))
    if trace:
        try:
            import ntff_shim
            ntff_shim.install()
        except Exception:
            trace = False

    h = np.ascontiguousarray(np.asarray(h, dtype=np.float32))
    W = np.ascontiguousarray(np.asarray(W, dtype=np.float32))
    a = np.ascontiguousarray(np.asarray(a, dtype=np.float32)).reshape(2 * OUT_DIM)
    row = np.asarray(row).astype(np.int64)
    col = np.asarray(col).astype(np.int64)

    # ---- pass 1: Wh / s_src / s_dst, node-sharded ----
    nc1 = _build_pass1()
    WT = np.ascontiguousarray(W.T)
    a2 = np.ascontiguousarray(np.stack([a[:OUT_DIM], a[OUT_DIM:]], axis=1))
    in_maps1 = []
    for c in range(CORES):
        hpad = np.zeros((NPP, IN_DIM), np.float32)
        hpad[:NPC] = h[c * NPC:(c + 1) * NPC]
        in_maps1.append({"hT": np.ascontiguousarray(hpad.T), "Wm": W,
                         "WT": WT, "a2": a2})
    res1 = _run_spmd(nc1, in_maps1, trace=trace)
    if trace:
        LAST_STATS["pass1_ns"] = res1.exec_time_ns

    WhA = np.ones((N_NODES, OUT_DIM + 1), np.float32)
    s_src = np.empty(N_NODES, np.float32)
    s_dst = np.empty(N_NODES, np.float32)
    for c in range(CORES):
        whT = res1.results[c]["whT"]
        WhA[c * NPC:(c + 1) * NPC, :OUT_DIM] = whT[:OUT_DIM, :NPC].T
        s_src[c * NPC:(c + 1) * NPC] = whT[OUT_DIM, :NPC]
        s_dst[c * NPC:(c + 1) * NPC] = whT[OUT_DIM + 1, :NPC]

    # ---- host: edge-slot structure + replicated-Wh message streams ----
    st = _prep_structure(row, col)
    Tj, Ttot = st["Tj"], st["Ttot"]
    W65 = OUT_DIM + 1
    cs, ps, tg = st["core_s"], st["p_s"], st["tglob"]
    row_s = row[st["order"]]
    col_s = col[st["order"]]

    msg = np.zeros((CORES, 128, Ttot, W65), np.float32)
    msg[cs, ps, tg] = WhA[col_s]
    qs = np.full((CORES, 128, Ttot), PAD_Q, np.float32)
    qs[cs, ps, tg] = s_src[row_s]
    qd = np.full((CORES, 128, Ttot), PAD_Q, np.float32)
    qd[cs, ps, tg] = s_dst[col_s]
    rr = np.zeros((CORES, 128, Ttot), np.float32)
    rr[cs, ps, tg] = (row_s - st["gb_s"] * BW).astype(np.float32)

    # ---- pass 2: attention + segment sum ----
    eps_free = int(np.bincount(row, minlength=N_NODES).min()) > 0
    nc2 = _build_pass2(Tj, Ttot, eps_free=eps_free)
    in_maps2 = [{"msg": msg[c].reshape(128, Ttot * W65),
                 "qrr": np.concatenate([qs[c], qd[c], rr[c]], axis=1)}
                for c in range(CORES)]
    res2 = _run_spmd(nc2, in_maps2, trace=trace)
    if trace:
        LAST_STATS["pass2_ns"] = res2.exec_time_ns
        LAST_STATS["total_ns"] = (res1.exec_time_ns or 0) + (res2.exec_time_ns or 0)

    out = np.empty((N_NODES, OUT_DIM), np.float32)
    sorted_ids = st["sorted_ids"]
    NGB_REAL = (N_NODES + BW - 1) // BW
    for c in range(CORES):
        dev = res2.results[c]["out"]
        for j in range(BPC):
            g = int(sorted_ids[j * CORES + c])
            if g >= NGB_REAL:
                continue
            sz = min(BW, N_NODES - g * BW)
            out[g * BW:g * BW + sz] = dev[j * BW:j * BW + sz]
    return out


# revision 22
# speedup vs baseline: 2.4342x; 1.0463x over previous
"""GAT influence layer on 8 Trainium2 NeuronCores (Bass/Tile).

Strategy (edge-parallel, row-sharded):
  Pass 1 (device): each core computes its 12.5k-node slice of
      Wh = h @ W, s_src = Wh @ a_src, s_dst = Wh @ a_dst
      via TensorE matmuls against an augmented weight matrix.
  Host: replicates/permutes device-computed Wh into per-core edge-slot
      streams (edges bucketed by destination-node block, 128-edge tiles),
      plus per-slot q_src/q_dst/row_rel arrays.  Data movement only.
  Pass 2 (device): one-shot exp(leakyrelu(q_src+q_dst)); per superblock a
      batched exp-weighted one-hot selection matrix (two DVE tensor_tensor
      ops with broadcast APs); the softmax-weighted segment-sum as
      PSUM-accumulated TensorE matmuls over the message stream; deferred
      division by the per-node denominator (the global max-subtract of the
      reference cancels analytically in the softmax).
  Host: concatenates per-core node-partitioned outputs.
"""

import os
import numpy as np

N_NODES = 100000
N_EDGES = 1600000
IN_DIM = 128
OUT_DIM = 64
NEG_SLOPE = 0.2
CORES = 8
NPC = N_NODES // CORES          # nodes per core (12500)
BW = 48                         # nodes per block (matmul window)
BPC = 261                       # block slots per core (8*261 >= ceil(N/48))
NPP2 = BPC * BW                 # padded nodes per core, pass 2 (12528)
NPP = 12544                     # padded nodes per core, pass 1 (98*128)
RT = NPP // 128                 # pass-1 row tiles per core (98)
SBB = 9                         # blocks per superblock (261/9 = 29 sbs)
PAD_Q = -30000.0                # pad-slot attention logit -> exp == 0

LAST_STATS = {}


def _build_pass1():
    from concourse import bacc, mybir
    import concourse.tile as tile

    f32 = mybir.dt.float32
    nc = bacc.Bacc("TRN2", target_bir_lowering=False, debug=False)
    d_hT = nc.dram_tensor("hT", [128, NPP], f32, kind="ExternalInput")
    d_W = nc.dram_tensor("Wm", [IN_DIM, OUT_DIM], f32, kind="ExternalInput")
    d_WT = nc.dram_tensor("WT", [OUT_DIM, IN_DIM], f32, kind="ExternalInput")
    d_a2 = nc.dram_tensor("a2", [OUT_DIM, 2], f32, kind="ExternalInput")
    d_whT = nc.dram_tensor("whT", [OUT_DIM + 2, NPP], f32, kind="ExternalOutput")

    NW = 512                    # moving-operand width (fp32 max)
    nck = (NPP + NW - 1) // NW  # 25 chunks (last partial: 256)
    with tile.TileContext(nc) as tc:
        with tc.tile_pool(name="c1", bufs=1) as cp, \
             tc.tile_pool(name="ht1", bufs=4) as hp, \
             tc.tile_pool(name="wo1", bufs=4) as wo, \
             tc.tile_pool(name="psw", bufs=1, space="PSUM") as psw, \
             tc.tile_pool(name="ps1", bufs=6, space="PSUM") as psp:
            w_sb = cp.tile([IN_DIM, OUT_DIM], f32)
            nc.sync.dma_start(out=w_sb[:], in_=d_W[:])
            wt_sb = cp.tile([OUT_DIM, IN_DIM], f32)
            nc.sync.dma_start(out=wt_sb[:], in_=d_WT[:])
            a_sb = cp.tile([OUT_DIM, 2], f32)
            nc.sync.dma_start(out=a_sb[:], in_=d_a2[:])

            waug = cp.tile([IN_DIM, OUT_DIM + 2], f32)
            nc.vector.tensor_copy(out=waug[:, 0:OUT_DIM], in_=w_sb[:])
            ws_ps = psw.tile([IN_DIM, 2], f32, space="PSUM")
            nc.tensor.matmul(out=ws_ps[:], lhsT=wt_sb[:], rhs=a_sb[:],
                             start=True, stop=True)
            nc.vector.tensor_copy(out=waug[:, OUT_DIM:OUT_DIM + 2], in_=ws_ps[:])

            CHW = 6 * NW        # 3072-col chunks: 1.5MB in-DMA, 0.8MB out-DMA
            for g0 in range(0, NPP, CHW):
                g1 = min(g0 + CHW, NPP)
                gw = g1 - g0
                ht = hp.tile([128, CHW], f32, tag="ht")
                nc.sync.dma_start(out=ht[:, :gw], in_=d_hT[:, g0:g1])
                wh_sb = wo.tile([OUT_DIM + 2, CHW], f32, tag="wh")
                for c0 in range(0, gw, NW):
                    w = min(c0 + NW, gw) - c0
                    wh_ps = psp.tile([OUT_DIM + 2, NW], f32, space="PSUM")
                    nc.tensor.matmul(out=wh_ps[:, :w], lhsT=waug[:],
                                     rhs=ht[:, c0:c0 + w], start=True, stop=True)
                    nc.vector.tensor_copy(out=wh_sb[:, c0:c0 + w], in_=wh_ps[:, :w])
                nc.sync.dma_start(out=d_whT[:, g0:g1], in_=wh_sb[:, :gw])
    nc.compile()
    return nc


def _build_pass2(Tj, Ttot, eps_free=False):
    from concourse import bacc, mybir
    import concourse.tile as tile

    f32 = mybir.dt.float32
    i32 = mybir.dt.int32
    alu = mybir.AluOpType
    act = mybir.ActivationFunctionType
    W65 = OUT_DIM + 1

    base = np.zeros(BPC + 1, np.int64)
    base[1:] = np.cumsum(Tj)
    assert base[-1] == Ttot

    nc = bacc.Bacc("TRN2", target_bir_lowering=False, debug=False)
    d_msg = nc.dram_tensor("msg", [128, Ttot * W65], f32, kind="ExternalInput")
    d_qrr = nc.dram_tensor("qrr", [128, 3 * Ttot], f32, kind="ExternalInput")
    d_out = nc.dram_tensor("out", [NPP2, OUT_DIM], f32, kind="ExternalOutput")

    n_sb = BPC // SBB
    with tile.TileContext(nc) as tc:
        with tc.tile_pool(name="c2", bufs=1) as cp, \
             tc.tile_pool(name="gp", bufs=4) as gp, \
             tc.tile_pool(name="mp", bufs=4) as mp, \
             tc.tile_pool(name="fp", bufs=8) as fp, \
             tc.tile_pool(name="op", bufs=3) as op, \
             tc.tile_pool(name="pp", bufs=8, space="PSUM") as pp:

            iota_i = cp.tile([128, BW], i32)
            nc.gpsimd.iota(iota_i[:], pattern=[[1, BW]], base=0, channel_multiplier=0)
            iota_f = cp.tile([128, BW], f32)
            nc.vector.tensor_copy(out=iota_f[:], in_=iota_i[:])

            qrr_sb = cp.tile([128, 3 * Ttot], f32)
            nc.sync.dma_start(out=qrr_sb[:], in_=d_qrr[:])
            qs_sb = qrr_sb[:, 0:Ttot]
            qd_sb = qrr_sb[:, Ttot:2 * Ttot]
            rr_sb = qrr_sb[:, 2 * Ttot:3 * Ttot]

            # exp(leakyrelu(qs + qd)); first superblock's slots first so the
            # pipeline can start while the rest computes
            ex_sb = cp.tile([128, Ttot], f32)
            sc_sb = cp.tile([128, Ttot], f32)
            Tcut = int(base[SBB])
            for a, b in ((0, Tcut), (Tcut, Ttot)):
                nc.vector.tensor_tensor(out=ex_sb[:, a:b], in0=qs_sb[:, a:b],
                                        in1=qd_sb[:, a:b], op=alu.add)
                nc.vector.tensor_scalar(out=sc_sb[:, a:b], in0=ex_sb[:, a:b],
                                        scalar1=NEG_SLOPE, scalar2=None, op0=alu.mult)
                nc.vector.tensor_tensor(out=ex_sb[:, a:b], in0=ex_sb[:, a:b],
                                        in1=sc_sb[:, a:b], op=alu.max)
                nc.scalar.activation(out=ex_sb[:, a:b], in_=ex_sb[:, a:b], func=act.Exp)

            for s in range(n_sb):
                j0, j1 = s * SBB, (s + 1) * SBB
                t0, t1 = int(base[j0]), int(base[j1])
                T_s = t1 - t0
                G = gp.tile([128, T_s * W65], f32, tag="G")
                nc.sync.dma_start(out=G[:], in_=d_msg[:, t0 * W65:t1 * W65])

                # M[p, (t,j)] = exp[p,t] * (iota_j == rrel[p,t]) — batched
                M = mp.tile([128, T_s * BW], f32, tag="M")
                eng = nc.vector
                eng.tensor_tensor(
                    out=M[:],
                    in0=iota_f[:].rearrange("p (o f) -> p o f", o=1).to_broadcast([128, T_s, BW]),
                    in1=rr_sb[:, t0:t1].rearrange("p (t o) -> p t o", o=1).to_broadcast([128, T_s, BW]),
                    op=alu.is_equal)
                eng.tensor_tensor(
                    out=M[:], in0=M[:],
                    in1=ex_sb[:, t0:t1].rearrange("p (t o) -> p t o", o=1).to_broadcast([128, T_s, BW]),
                    op=alu.mult)

                out_stage = op.tile([BW, SBB * OUT_DIM], f32, tag="ost")
                for j in range(j0, j1):
                    tj = int(Tj[j])
                    ps = pp.tile([BW, W65], f32, space="PSUM", tag="ps")
                    for t in range(tj):
                        rel = int(base[j]) + t - t0
                        nc.tensor.matmul(out=ps[:],
                                         lhsT=M[:, rel * BW:(rel + 1) * BW],
                                         rhs=G[:, rel * W65:(rel + 1) * W65],
                                         start=(t == 0), stop=(t == tj - 1))
                    dinv = fp.tile([BW, 1], f32, tag="di")
                    if eps_free:
                        nc.vector.reciprocal(out=dinv[:], in_=ps[:, OUT_DIM:W65])
                    else:
                        dtmp = fp.tile([BW, 1], f32, tag="dt")
                        nc.vector.tensor_scalar(out=dtmp[:], in0=ps[:, OUT_DIM:W65],
                                                scalar1=1e-10, scalar2=None, op0=alu.add)
                        nc.vector.reciprocal(out=dinv[:], in_=dtmp[:])
                    jr = j - j0
                    nc.scalar.activation(out=out_stage[:, jr * OUT_DIM:(jr + 1) * OUT_DIM],
                                         in_=ps[:, 0:OUT_DIM], func=act.Copy,
                                         scale=dinv[:])
                out_ap = d_out[j0 * BW:j1 * BW, :].rearrange("(b p) f -> p b f", p=BW)
                in_ap = out_stage[:].rearrange("p (b f) -> p b f", b=SBB)
                nc.sync.dma_start(out=out_ap, in_=in_ap)
    nc.compile()
    return nc


def _prep_structure(row, col):
    """Bucket edges by dest-node 64-block; permute blocks onto (core, slot)
    pairs so that blocks sharing a slot index have similar edge counts
    (shrinks the shared per-slot tile count); assign each edge a slot
    (partition p, tile column t) in its block's 128-edge tiles."""
    NGB = CORES * BPC                       # 1568 block slots (1563 real)
    gb = row // BW                          # global 64-node block per edge
    cnt = np.bincount(gb, minlength=NGB)
    sorted_ids = np.argsort(-cnt, kind="stable")
    blk_core = np.empty(NGB, np.int64)
    blk_slot = np.empty(NGB, np.int64)
    k = np.arange(NGB)
    blk_core[sorted_ids] = k % CORES
    blk_slot[sorted_ids] = k // CORES
    # per slot j: max count over its 8 assigned blocks (sorted -> first of 8)
    Tj = np.maximum(1, (cnt[sorted_ids[::CORES]] + 127) // 128)
    base = np.zeros(BPC + 1, np.int64)
    base[1:] = np.cumsum(Tj)
    Ttot = int(base[-1])

    key = blk_core[gb] * BPC + blk_slot[gb]
    kcnt = np.bincount(key, minlength=NGB)
    order = np.argsort(key, kind="stable")
    starts = np.zeros(NGB, np.int64)
    starts[1:] = np.cumsum(kcnt)[:-1]
    rank = np.arange(N_EDGES, dtype=np.int64) - np.repeat(starts, kcnt)
    key_s = key[order]
    core_s = key_s // BPC
    slot_s = key_s - core_s * BPC
    t_loc = rank >> 7
    p_s = rank & 127
    tglob = base[slot_s] + t_loc
    return dict(order=order, core_s=core_s, p_s=p_s, tglob=tglob,
                gb_s=gb[order], Tj=Tj, base=base, Ttot=Ttot,
                sorted_ids=sorted_ids)


def _run_spmd(nc, in_maps, trace=False):
    from concourse import bass_utils
    res = bass_utils.run_bass_kernel_spmd(
        nc, in_maps, core_ids=list(range(CORES)), trace=trace)
    return res


def kernel(h, row, col, W, a):
    trace = bool(os.environ.get("GAT_TRACE"))
    if trace:
        try:
            import ntff_shim
            ntff_shim.install()
        except Exception:
            trace = False

    h = np.ascontiguousarray(np.asarray(h, dtype=np.float32))
    W = np.ascontiguousarray(np.asarray(W, dtype=np.float32))
    a = np.ascontiguousarray(np.asarray(a, dtype=np.float32)).reshape(2 * OUT_DIM)
    row = np.asarray(row).astype(np.int64)
    col = np.asarray(col).astype(np.int64)

    # ---- pass 1: Wh / s_src / s_dst, node-sharded ----
    nc1 = _build_pass1()
    WT = np.ascontiguousarray(W.T)
    a2 = np.ascontiguousarray(np.stack([a[:OUT_DIM], a[OUT_DIM:]], axis=1))
    in_maps1 = []
    for c in range(CORES):
        hpad = np.zeros((NPP, IN_DIM), np.float32)
        hpad[:NPC] = h[c * NPC:(c + 1) * NPC]
        in_maps1.append({"hT": np.ascontiguousarray(hpad.T), "Wm": W,
                         "WT": WT, "a2": a2})
    res1 = _run_spmd(nc1, in_maps1, trace=trace)
    if trace:
        LAST_STATS["pass1_ns"] = res1.exec_time_ns

    WhA = np.ones((N_NODES, OUT_DIM + 1), np.float32)
    s_src = np.empty(N_NODES, np.float32)
    s_dst = np.empty(N_NODES, np.float32)
    for c in range(CORES):
        whT = res1.results[c]["whT"]
        WhA[c * NPC:(c + 1) * NPC, :OUT_DIM] = whT[:OUT_DIM, :NPC].T
        s_src[c * NPC:(c + 1) * NPC] = whT[OUT_DIM, :NPC]
        s_dst[c * NPC:(c + 1) * NPC] = whT[OUT_DIM + 1, :NPC]

    # ---- host: edge-slot structure + replicated-Wh message streams ----
    st = _prep_structure(row, col)
    Tj, Ttot = st["Tj"], st["Ttot"]
    W65 = OUT_DIM + 1
    cs, ps, tg = st["core_s"], st["p_s"], st["tglob"]
    row_s = row[st["order"]]
    col_s = col[st["order"]]

    msg = np.zeros((CORES, 128, Ttot, W65), np.float32)
    msg[cs, ps, tg] = WhA[col_s]
    qs = np.full((CORES, 128, Ttot), PAD_Q, np.float32)
    qs[cs, ps, tg] = s_src[row_s]
    qd = np.full((CORES, 128, Ttot), PAD_Q, np.float32)
    qd[cs, ps, tg] = s_dst[col_s]
    rr = np.zeros((CORES, 128, Ttot), np.float32)
    rr[cs, ps, tg] = (row_s - st["gb_s"] * BW).astype(np.float32)

    # ---- pass 2: attention + segment sum ----
    eps_free = int(np.bincount(row, minlength=N_NODES).min()) > 0
    nc2 = _build_pass2(Tj, Ttot, eps_free=eps_free)
    in_maps2 = [{"msg": msg[c].reshape(128, Ttot * W65),
                 "qrr": np.concatenate([qs[c], qd[c], rr[c]], axis=1)}
                for c in range(CORES)]
    res2 = _run_spmd(nc2, in_maps2, trace=trace)
    if trace:
        LAST_STATS["pass2_ns"] = res2.exec_time_ns
        LAST_STATS["total_ns"] = (res1.exec_time_ns or 0) + (res2.exec_time_ns or 0)

    out = np.empty((N_NODES, OUT_DIM), np.float32)
    sorted_ids = st["sorted_ids"]
    NGB_REAL = (N_NODES + BW - 1) // BW
    for c in range(CORES):
        dev = res2.results[c]["out"]
        for j in range(BPC):
            g = int(sorted_ids[j * CORES + c])
            if g >= NGB_REAL:
                continue
            sz = min(BW, N_NODES - g * BW)
            out[g * BW:g * BW + sz] = dev[j * BW:j * BW + sz]
    return out
